# revision 2
# baseline (speedup 1.0000x reference)
"""Trainium2 Bass kernel for a 2-layer GCN (FCGraphGNN) over 8 NeuronCores.

Math (matches reference):
  ew' = [edge_attr; ones(N)]  (self loops), deg = segsum(ew', dst), dinv = deg^-1/2
  h1 = relu(segsum(dinv[src]*ew*dinv[dst] * (x@W1)[src]) + b1)
  h2 = relu(segsum(norm * (h1@W2)[src]) + b2)
  out = mean-pool-by-graph(h2) @ Wo + bo

Strategy:
  - Shard edges by dst across 8 cores (sorted by dst host-side).
  - Virtual node space: nodes packed into windows of <=64 consecutive nodes,
    each window has a fixed [T_SIDE lo-tiles | T_SIDE hi-tiles] slot layout so
    one SPMD program serves all cores (different data, same shapes).
  - Messages gathered with dma_gather (256B rows) from a per-core DRAM table
    H row-scaled by dinv[src]; dst-side dinv folded into the window epilogue.
  - Per 128-edge tile: DVE builds S = (iota==dst_slot)*ew; PE accumulates
    S.T @ M into PSUM per window (the segment-sum).
  - dinv + H1 all-gathered across cores; pooled partial sums all-reduced.
"""

import os
import sys

import numpy as np

sys.path.insert(0, "/opt/trn_rl_repo")

# ---------------------------------------------------------------- constants
N_NODES = 50000
N_EDGES = 3200000
N_GRAPHS = 50
IN_F = 5
HID = 64
OUT_F = 2
N_CORES = 8

SLOTS = 64          # dst nodes per window
T_SIDE = 17         # 128-edge tiles per (window, src-half)
CAP = T_SIDE * 128  # edge slots per (window, side)
GROUP_W = 4         # windows fetched per dma_gather pair
SG = 52             # graph one-hot width (50 graphs + 2 junk bins)


def _pack_host(x, edge_index, edge_attr, batch):
    """Pure index/layout preprocessing (numpy). Returns per-core input dicts
    plus the static plan (NW, DEG_K...)."""
    src = np.asarray(edge_index[0], dtype=np.int64)
    dst = np.asarray(edge_index[1], dtype=np.int64)
    ew = np.asarray(edge_attr, dtype=np.float32).reshape(-1)
    loop = np.arange(N_NODES, dtype=np.int64)
    src = np.concatenate([src, loop]).astype(np.int32)
    dst = np.concatenate([dst, loop]).astype(np.int32)
    ew = np.concatenate([ew, np.ones(N_NODES, np.float32)])
    E = src.shape[0]

    deg_cnt = np.bincount(dst, minlength=N_NODES).astype(np.int64)
    node_ptr = np.zeros(N_NODES + 1, np.int64)
    np.cumsum(deg_cnt, out=node_ptr[1:])
    order = np.argsort(dst, kind="stable")

    # core node boundaries balancing edge counts
    cum = node_ptr[1:]
    nb = [0]
    for c in range(1, N_CORES):
        nb.append(int(np.searchsorted(cum, c * E / N_CORES)))
    nb.append(N_NODES)
    nb = np.array(nb, np.int64)
    split_node = int(nb[4])  # src < split_node -> "lo" half of virtual space

    side_lo = src < split_node
    deg_lo = np.bincount(dst[side_lo], minlength=N_NODES).astype(np.int64)
    deg_hi = deg_cnt - deg_lo

    # window packing per core
    core_windows = []
    for c in range(N_CORES):
        wlist = []
        v = int(nb[c])
        end = int(nb[c + 1])
        while v < end:
            ws = v
            lo = hi = cnt = 0
            while (
                v < end
                and cnt < SLOTS
                and lo + deg_lo[v] <= CAP
                and hi + deg_hi[v] <= CAP
            ):
                lo += int(deg_lo[v])
                hi += int(deg_hi[v])
                cnt += 1
                v += 1
            wlist.append((ws, v))
        core_windows.append(wlist)

    NW = max(len(w) for w in core_windows)
    NW = (NW + 7) // 8 * 8  # multiple of GROUP_W and the h0-write batch
    assert NW <= 128, f"NW={NW} exceeds int16 index budget"
    NVC = NW * SLOTS
    NV = N_CORES * NVC
    NVH = NV // 2

    # vid map (node -> virtual id)
    node_vid = np.zeros(N_NODES, np.int32)
    for c in range(N_CORES):
        for w, (ws, we) in enumerate(core_windows[c]):
            node_vid[ws:we] = c * NVC + w * SLOTS + np.arange(we - ws, dtype=np.int32)

    DEG_K = int(deg_cnt.max())
    DEG_K = (DEG_K + 3) // 4 * 4

    # per-side dst-sorted edge lists + ptrs
    lo_edges = order[side_lo[order]]
    hi_edges = order[~side_lo[order]]
    lo_ptr = np.zeros(N_NODES + 1, np.int64)
    np.cumsum(deg_lo, out=lo_ptr[1:])
    hi_ptr = np.zeros(N_NODES + 1, np.int64)
    np.cumsum(deg_hi, out=hi_ptr[1:])

    NTILES = NW * 2 * T_SIDE
    NG = NW // GROUP_W
    GI = GROUP_W * CAP          # idxs per gather call
    IDXC = GI // 16

    vid_src = node_vid[src]

    # global position of each edge within its dst node's sorted run
    col_within = np.empty(E, np.int64)
    ar = np.arange(E, dtype=np.int64)
    col_within[order] = ar - node_ptr[dst[order]]

    # xt in virtual layout (shared by all cores)
    xt_virt = np.zeros((IN_F, NV), np.float32)
    xt_virt[:, node_vid] = np.asarray(x, np.float32).T

    batch_i = np.asarray(batch, np.int64)

    per_core = []
    for c in range(N_CORES):
        wlist = core_windows[c]
        ewp = np.zeros((NTILES, 128), np.float32)
        dstp = np.zeros((NTILES, 128), np.float32)
        idx_lo = np.zeros((NW, CAP), np.int16)
        idx_hi = np.zeros((NW, CAP), np.int16)
        gid = np.full((SLOTS, NW), 50.0, np.float32)
        ewdeg = np.zeros((NVC, DEG_K), np.float32)

        for w, (ws, we) in enumerate(wlist):
            for s, (edges, ptr, idxbuf, voff) in enumerate(
                ((lo_edges, lo_ptr, idx_lo, 0), (hi_edges, hi_ptr, idx_hi, NVH))
            ):
                ids = edges[ptr[ws] : ptr[we]]
                n = ids.shape[0]
                t0 = (w * 2 + s) * T_SIDE
                flat_ew = ewp.reshape(-1)
                flat_dst = dstp.reshape(-1)
                base = t0 * 128
                flat_ew[base : base + n] = ew[ids]
                flat_dst[base : base + n] = (dst[ids] - ws).astype(np.float32)
                idxbuf[w, :n] = (vid_src[ids] - voff).astype(np.int16)
            gid[: we - ws, w] = batch_i[ws:we].astype(np.float32)

        # padded per-node edge weights for the degree pass
        e_lo = int(node_ptr[nb[c]])
        e_hi = int(node_ptr[nb[c + 1]])
        es = order[e_lo:e_hi]
        rows = node_vid[dst[es]] - c * NVC
        ewdeg[rows, col_within[es]] = ew[es]
        rowdeg = np.zeros(NVC, np.int64)
        nr = node_vid[nb[c] : nb[c + 1]] - c * NVC
        rowdeg[nr] = deg_cnt[nb[c] : nb[c + 1]]
        ewdeg[rowdeg == 0, 0] = 1.0

        # wrap gather indices: [NG, 128, IDXC] (16-partition wrap, replicated)
        def wrap(a):
            g = a.reshape(NG, GI // 16, 16).transpose(0, 2, 1)  # [NG,16,IDXC]
            return np.ascontiguousarray(np.tile(g, (1, 8, 1)))

        per_core.append(
            dict(
                ewcols=np.ascontiguousarray(ewp.T),
                dstcols=np.ascontiguousarray(dstp.T),
                idxlo=wrap(idx_lo.reshape(-1)),
                idxhi=wrap(idx_hi.reshape(-1)),
                gid=np.ascontiguousarray(gid),
                ewdeg=ewdeg,
            )
        )

    plan = dict(
        NW=NW, NVC=NVC, NV=NV, NVH=NVH, DEG_K=DEG_K,
        NTILES=NTILES, NG=NG, GI=GI, IDXC=IDXC,
    )
    return per_core, plan, xt_virt


def _build_program(plan):
    import concourse.bacc as bacc
    import concourse.bass as bass
    import concourse.tile as tile
    from concourse import mybir
    from concourse.tile_rust import add_dep_helper

    f32 = mybir.dt.float32
    i16 = mybir.dt.int16
    Alu = mybir.AluOpType
    Act = mybir.ActivationFunctionType

    NW = plan["NW"]; NVC = plan["NVC"]; NV = plan["NV"]; NVH = plan["NVH"]
    DEG_K = plan["DEG_K"]; NTILES = plan["NTILES"]; NG = plan["NG"]
    GI = plan["GI"]; IDXC = plan["IDXC"]

    STAGE = int(os.environ.get("KSTAGE", "9"))
    NQ = int(os.environ.get("KNQ", "4"))
    nc = bacc.Bacc("TRN2", target_bir_lowering=False, debug=False,
                   num_devices=N_CORES, num_swdge_queues=NQ)

    xt = nc.declare_dram_parameter("xt", [IN_F, NV], f32, isOutput=False)
    w1 = nc.declare_dram_parameter("w1", [IN_F, HID], f32, isOutput=False)
    w2 = nc.declare_dram_parameter("w2", [HID, HID], f32, isOutput=False)
    wo = nc.declare_dram_parameter("wo", [HID, OUT_F], f32, isOutput=False)
    b1 = nc.declare_dram_parameter("b1", [SLOTS, HID], f32, isOutput=False)
    b2 = nc.declare_dram_parameter("b2", [SLOTS, HID], f32, isOutput=False)
    bo = nc.declare_dram_parameter("bo", [N_GRAPHS, OUT_F], f32, isOutput=False)
    ewdeg = nc.declare_dram_parameter("ewdeg", [NVC, DEG_K], f32, isOutput=False)
    ewcols = nc.declare_dram_parameter("ewcols", [128, NTILES], f32, isOutput=False)
    dstcols = nc.declare_dram_parameter("dstcols", [128, NTILES], f32, isOutput=False)
    idxlo = nc.declare_dram_parameter("idxlo", [NG, 128, IDXC], i16, isOutput=False)
    idxhi = nc.declare_dram_parameter("idxhi", [NG, 128, IDXC], i16, isOutput=False)
    gidp = nc.declare_dram_parameter("gid", [SLOTS, NW], f32, isOutput=False)
    out = nc.declare_dram_parameter("out", [N_GRAPHS, OUT_F], f32, isOutput=True)
    chain_in = nc.declare_dram_parameter("chain", [1, 4], f32, isOutput=False)
    chain_out = nc.declare_dram_parameter("chain_out", [1, 4], f32, isOutput=True)
    KDBG = int(os.environ.get("KDBG", "0"))
    if KDBG:
        dbg_dinv = nc.declare_dram_parameter("dbg_dinv", [SLOTS, NW], f32, isOutput=True)
        dbg_h0 = nc.declare_dram_parameter("dbg_h0", [NV, HID], f32, isOutput=True)
        dbg_h1 = nc.declare_dram_parameter("dbg_h1", [2048, HID], f32, isOutput=True)
        dbg_pool = nc.declare_dram_parameter("dbg_pool", [HID + 1, SG], f32, isOutput=True)
        dbg_mlo = nc.declare_dram_parameter("dbg_mlo", [128, 68, HID], f32, isOutput=True)

    groups = [list(range(N_CORES))]

    with tile.TileContext(nc) as tc:
        with (
            tc.tile_pool(name="dram", bufs=1, space="DRAM") as dram,
            tc.tile_pool(name="const", bufs=1) as cpool,
            tc.tile_pool(name="persist", bufs=1) as ppool,
        ):
            h0 = dram.tile([NV, HID], f32, tag="h0")
            h1loc = dram.tile([NVC, HID], f32, tag="h1loc")
            h1glob = dram.tile([NV, HID], f32, tag="h1glob")
            dinv_loc_d = dram.tile([SLOTS, NW], f32, tag="dinvloc")
            dinv_glob_d = dram.tile([N_CORES, SLOTS, NW], f32, tag="dinvglob")
            pool_in_d = dram.tile([HID + 1, SG], f32, tag="poolin")
            pool_out_d = dram.tile([HID + 1, SG], f32, tag="poolout")

            # ---- constants
            iota64 = cpool.tile([128, SLOTS], f32, tag="iota64")
            nc.gpsimd.iota(iota64[:], pattern=[[1, SLOTS]], base=0,
                           channel_multiplier=0,
                           allow_small_or_imprecise_dtypes=True)
            iota52 = cpool.tile([SLOTS, SG], f32, tag="iota52")
            nc.gpsimd.iota(iota52[:], pattern=[[1, SG]], base=0,
                           channel_multiplier=0,
                           allow_small_or_imprecise_dtypes=True)
            w1s = cpool.tile([IN_F, HID], f32, tag="w1s")
            nc.sync.dma_start(w1s[:], w1[:])
            w2s = cpool.tile([HID, HID], f32, tag="w2s")
            nc.sync.dma_start(w2s[:], w2[:])
            wos = cpool.tile([HID, OUT_F], f32, tag="wos")
            nc.sync.dma_start(wos[:], wo[:])
            b1s = cpool.tile([SLOTS, HID], f32, tag="b1s")
            nc.sync.dma_start(b1s[:], b1[:])
            b2s = cpool.tile([SLOTS, HID], f32, tag="b2s")
            nc.sync.dma_start(b2s[:], b2[:])
            bos = cpool.tile([N_GRAPHS, OUT_F], f32, tag="bos")
            nc.sync.dma_start(bos[:], bo[:])
            gids = cpool.tile([SLOTS, NW], f32, tag="gids")
            nc.sync.dma_start(gids[:], gidp[:])
            ewc = cpool.tile([128, NTILES], f32, tag="ewc")
            nc.sync.dma_start(ewc[:], ewcols[:])
            dstc = cpool.tile([128, NTILES], f32, tag="dstc")
            nc.sync.dma_start(dstc[:], dstcols[:])

            dinvw = ppool.tile([SLOTS, NW], f32, tag="dinvw")
            dinvg = ppool.tile([SLOTS, N_CORES, NW], f32, tag="dinvg")

            KAMP = int(os.environ.get("KAMP", "1"))
            KCC = int(os.environ.get("KCC", "1"))
            KNG = int(os.environ.get("KNG", "9999"))
            KGATHER = int(os.environ.get("KGATHER", "1"))

            # ---- message-passing layer (one pass over the edge tiles)
            def layer(l, rep, src_table, fence):
                lo_view = src_table[0:NVH, :]
                hi_view = src_table[NVH:NV, :]
                with (
                    tc.tile_pool(name=f"idx{l}_{rep}", bufs=4) as ipool,
                    tc.tile_pool(name=f"mbuf{l}_{rep}", bufs=2) as mpool,
                    tc.tile_pool(name=f"sbld{l}_{rep}", bufs=6) as spool,
                    tc.tile_pool(name=f"wpsum{l}_{rep}", bufs=4, space="PSUM") as wpool,
                    tc.tile_pool(name=f"epi{l}_{rep}", bufs=3) as epool,
                    tc.tile_pool(name=f"p2_{l}_{rep}", bufs=2, space="PSUM") as p2pool,
                    tc.tile_pool(name=f"gpool{l}_{rep}", bufs=1, space="PSUM") as gpool,
                ):
                    if l == 2:
                        pool_ps = gpool.tile([HID + 1, SG], f32, tag="poolps")
                    for g in range(min(NG, KNG)):
                        ilo = ipool.tile([128, IDXC], i16, tag="ilo")
                        nc.sync.dma_start(ilo[:], idxlo[g])
                        ihi = ipool.tile([128, IDXC], i16, tag="ihi")
                        nc.sync.dma_start(ihi[:], idxhi[g])
                        mlo = mpool.tile([128, GROUP_W * T_SIDE, HID], f32,
                                         tag="mlo")
                        mhi = mpool.tile([128, GROUP_W * T_SIDE, HID], f32,
                                         tag="mhi")
                        if KGATHER:
                            glo = nc.gpsimd.dma_gather(
                                mlo[:], lo_view, ilo[:], GI, GI, HID,
                                single_packet=False,
                                queue_num=(2 * g) % NQ,
                            )
                            ghi = nc.gpsimd.dma_gather(
                                mhi[:], hi_view, ihi[:], GI, GI, HID,
                                single_packet=False,
                                queue_num=(2 * g + 1) % NQ,
                            )
                            if fence is not None:
                                add_dep_helper(glo.ins, fence.ins,
                                               reason="gather src table ready")
                                add_dep_helper(ghi.ins, fence.ins,
                                               reason="gather src table ready")
                        else:
                            nc.sync.dma_start(
                                mlo[:],
                                src_table[0:GI, :].rearrange(
                                    "(b p) h -> p b h", p=128
                                ),
                            )
                            nc.sync.dma_start(
                                mhi[:],
                                src_table[0:GI, :].rearrange(
                                    "(b p) h -> p b h", p=128
                                ),
                            )
                        if KDBG and l == 1 and g == 0 and rep == 0:
                            nc.sync.dma_start(dbg_mlo[:], mlo[:])
                        for wl in range(GROUP_W):
                            w = g * GROUP_W + wl
                            ps = wpool.tile([SLOTS, HID], f32, tag="wps")
                            k = 0
                            for s, mb in ((0, mlo), (1, mhi)):
                                for ti in range(T_SIDE):
                                    t = (w * 2 + s) * T_SIDE + ti
                                    blk = wl * T_SIDE + ti
                                    S = spool.tile([128, SLOTS], f32, tag="S")
                                    nc.vector.tensor_scalar(
                                        out=S[:], in0=iota64[:],
                                        scalar1=dstc[:, t : t + 1],
                                        scalar2=ewc[:, t : t + 1],
                                        op0=Alu.is_equal, op1=Alu.mult,
                                    )
                                    first = k == 0
                                    last = k == 2 * T_SIDE - 1
                                    if l == 1:
                                        nc.tensor.matmul(
                                            out=ps[:], lhsT=S[:],
                                            rhs=mb[:, blk, :],
                                            start=first, stop=last,
                                        )
                                    else:
                                        nc.tensor.matmul(
                                            out=ps[:], lhsT=mb[:, blk, :],
                                            rhs=S[:],
                                            start=first, stop=last,
                                        )
                                    k += 1
                            dv = dinvw[:, w : w + 1]
                            if l == 1:
                                u = epool.tile([SLOTS, HID], f32, tag="u1")
                                nc.vector.tensor_scalar(
                                    out=u[:], in0=ps[:], scalar1=dv,
                                    scalar2=None, op0=Alu.mult,
                                )
                                nc.vector.tensor_tensor(
                                    out=u[:], in0=u[:], in1=b1s[:], op=Alu.add,
                                )
                                nc.vector.tensor_scalar(
                                    out=u[:], in0=u[:], scalar1=0.0,
                                    scalar2=None, op0=Alu.max,
                                )
                                uh = epool.tile([SLOTS, HID], f32, tag="uh")
                                nc.vector.tensor_scalar(
                                    out=uh[:], in0=u[:], scalar1=dv,
                                    scalar2=None, op0=Alu.mult,
                                )
                                nc.sync.dma_start(
                                    h1loc[w * SLOTS : (w + 1) * SLOTS, :],
                                    uh[:],
                                )
                            else:
                                aggT = epool.tile([HID, SLOTS], f32, tag="aggT")
                                nc.vector.tensor_copy(aggT[:], ps[:])
                                ps2 = p2pool.tile([SLOTS, HID], f32, tag="ps2")
                                nc.tensor.matmul(
                                    out=ps2[:], lhsT=aggT[:], rhs=w2s[:],
                                    start=True, stop=True,
                                )
                                u = epool.tile([SLOTS, HID + 1], f32, tag="u2")
                                nc.vector.memset(u[:, HID : HID + 1], 1.0)
                                nc.vector.tensor_scalar(
                                    out=u[:, 0:HID], in0=ps2[:], scalar1=dv,
                                    scalar2=None, op0=Alu.mult,
                                )
                                nc.vector.tensor_tensor(
                                    out=u[:, 0:HID], in0=u[:, 0:HID],
                                    in1=b2s[:], op=Alu.add,
                                )
                                nc.vector.tensor_scalar(
                                    out=u[:, 0:HID], in0=u[:, 0:HID],
                                    scalar1=0.0, scalar2=None, op0=Alu.max,
                                )
                                Sg = epool.tile([SLOTS, SG], f32, tag="Sg")
                                nc.vector.tensor_scalar(
                                    out=Sg[:], in0=iota52[:],
                                    scalar1=gids[:, w : w + 1],
                                    scalar2=None, op0=Alu.is_equal,
                                )
                                nc.tensor.matmul(
                                    out=pool_ps[:], lhsT=u[:], rhs=Sg[:],
                                    start=(w == 0),
                                    stop=(w == min(NG, KNG) * GROUP_W - 1),
                                )
                    if l == 2 and KNG >= NG:
                        pst = epool.tile([HID + 1, SG], f32, tag="pst")
                        nc.vector.tensor_copy(pst[:], pool_ps[:])
                        nc.sync.dma_start(pool_in_d[:], pst[:])

            # ---- one full pipeline iteration (repeated KAMP x for timing)
            for rep in range(KAMP):
                # degree pass -> local dinv
                with tc.tile_pool(name=f"deg{rep}", bufs=1) as dpool:
                    degt = dpool.tile([SLOTS, NW, DEG_K], f32, tag="degt")
                    nc.sync.dma_start(
                        degt[:], ewdeg[:].rearrange("(w s) k -> s w k", s=SLOTS)
                    )
                    deg = dpool.tile([SLOTS, NW], f32, tag="deg")
                    nc.vector.tensor_reduce(
                        out=deg[:], in_=degt[:], axis=mybir.AxisListType.X,
                        op=Alu.add,
                    )
                    rec = dpool.tile([SLOTS, NW], f32, tag="rec")
                    nc.vector.reciprocal(rec[:], deg[:])
                    nc.scalar.activation(dinvw[:], rec[:], Act.Sqrt)
                    nc.sync.dma_start(dinv_loc_d[:], dinvw[:])

                # all-gather dinv
                if KCC:
                    nc.gpsimd.collective_compute(
                        "AllGather", Alu.bypass, replica_groups=groups,
                        ins=[dinv_loc_d[:].rearrange("s w -> (s w)")],
                        outs=[dinv_glob_d[:].rearrange("r s w -> (r s w)")],
                    )
                else:
                    for _r in range(N_CORES):
                        nc.sync.dma_start(dinv_glob_d[_r], dinv_loc_d[:])
                nc.sync.dma_start(
                    dinvg[:], dinv_glob_d[:].rearrange("r s w -> s r w")
                )

                # preamble: h0 = dinv * (x @ W1), all rows, per core
                if STAGE >= 1:
                    with (
                        tc.tile_pool(name=f"pre{rep}", bufs=2) as prepool,
                        tc.tile_pool(name=f"prepsum{rep}", bufs=4,
                                     space="PSUM") as pspool,
                        tc.tile_pool(name=f"prestage{rep}", bufs=2) as stpool,
                    ):
                        BW = 8  # window tiles per h0 write
                        h0_writes = []
                        for r in range(N_CORES):
                            xts = prepool.tile([IN_F, NVC], f32, tag="xts")
                            nc.sync.dma_start(
                                xts[:], xt[:, r * NVC : (r + 1) * NVC]
                            )
                            for wb in range(NW // BW):
                                stage = stpool.tile([SLOTS, BW, HID], f32,
                                                    tag="h0st")
                                for j in range(BW):
                                    w = wb * BW + j
                                    ps = pspool.tile([SLOTS, HID], f32,
                                                     tag="prepsum")
                                    nc.tensor.matmul(
                                        out=ps[:],
                                        lhsT=xts[:, w * SLOTS : (w + 1) * SLOTS],
                                        rhs=w1s[:],
                                        start=True, stop=True,
                                    )
                                    nc.scalar.activation(
                                        stage[:, j, :], ps[:], Act.Copy,
                                        scale=dinvg[:, r, w : w + 1],
                                    )
                                row0 = (r * NW + wb * BW) * SLOTS
                                h0_writes.append(
                                    nc.sync.dma_start(
                                        h0[row0 : row0 + BW * SLOTS, :].rearrange(
                                            "(b s) h -> s b h", s=SLOTS
                                        ),
                                        stage[:],
                                    )
                                )

                # fence: gathers reading h0 wait on all its writes
                if STAGE >= 1:
                    fence0 = nc.gpsimd.engine_nop()
                    for wi in h0_writes:
                        add_dep_helper(fence0.ins, wi.ins,
                                       reason="h0 table ready before gathers")
                else:
                    fence0 = None

                if STAGE >= 2:
                    layer(1, rep, h0, fence0)

                if STAGE >= 3:
                    cc_h1 = nc.gpsimd.collective_compute(
                        "AllGather", Alu.bypass, replica_groups=groups,
                        ins=[h1loc[:].rearrange("a b -> (a b)")],
                        outs=[h1glob[:].rearrange("a b -> (a b)")],
                    )

                if STAGE >= 4:
                    layer(2, rep, h1glob, cc_h1)

            if STAGE < 4:
                # keep the tail runnable: zero the pooled partials
                with tc.tile_pool(name="dummy", bufs=1) as dpool2:
                    osb0 = dpool2.tile([HID + 1, SG], f32, tag="osb0")
                    nc.vector.memset(osb0[:], 0.0)
                    nc.sync.dma_start(pool_in_d[:], osb0[:])

            if KDBG:
                nc.sync.dma_start(dbg_dinv[:], dinvw[:])
                nc.sync.dma_start(dbg_h0[:], h0[:])
                nc.sync.dma_start(dbg_h1[:], h1loc[0:2048, :])
                nc.sync.dma_start(dbg_pool[:], pool_in_d[:])

            # ---- pooled partial sums -> all-reduce -> final linear
            if KCC:
                nc.gpsimd.collective_compute(
                    "AllReduce", Alu.add, replica_groups=groups,
                    ins=[pool_in_d[:]], outs=[pool_out_d[:]],
                )
            else:
                nc.sync.dma_start(pool_out_d[:], pool_in_d[:])
            with (
                tc.tile_pool(name="fin", bufs=1) as fpool,
                tc.tile_pool(name="finps", bufs=1, space="PSUM") as fpsum,
            ):
                pr = fpool.tile([HID + 1, SG], f32, tag="pr")
                nc.sync.dma_start(pr[:], pool_out_d[:])
                cm = fpool.tile([1, SG], f32, tag="cm")
                nc.vector.tensor_scalar(
                    out=cm[:], in0=pr[HID : HID + 1, :], scalar1=1.0,
                    scalar2=None, op0=Alu.max,
                )
                rcp = fpool.tile([1, SG], f32, tag="rcp")
                nc.vector.reciprocal(rcp[:], cm[:])
                rcpb = fpool.tile([HID, SG], f32, tag="rcpb")
                nc.gpsimd.partition_broadcast(rcpb[:], rcp[:])
                pooledT = fpool.tile([HID, N_GRAPHS], f32, tag="pooledT")
                nc.vector.tensor_tensor(
                    out=pooledT[:], in0=pr[0:HID, 0:N_GRAPHS],
                    in1=rcpb[0:HID, 0:N_GRAPHS],
                    op=Alu.mult,
                )
                pso = fpsum.tile([N_GRAPHS, OUT_F], f32, tag="pso")
                nc.tensor.matmul(
                    out=pso[:], lhsT=pooledT[:], rhs=wos[:],
                    start=True, stop=True,
                )
                osb = fpool.tile([N_GRAPHS, OUT_F], f32, tag="osb")
                nc.vector.tensor_tensor(
                    out=osb[:], in0=pso[:],
                    in1=bos[:],
                    op=Alu.add,
                )
                nc.sync.dma_start(out[:], osb[:])
                chs = fpool.tile([1, 4], f32, tag="chs")
                nc.sync.dma_start(chs[:], chain_in[:])
                nc.vector.tensor_scalar_add(chs[:], chs[:], 1.0)
                nc.sync.dma_start(chain_out[:], chs[:])

    nc.compile()
    return nc


def _make_runner(nc, repeat=1):
    """Cached-jit SPMD runner modeled on bass2jax.run_bass_via_pjrt, for
    benchmarking: returns (fn, prep) where prep(in_maps) device-puts inputs
    once and fn(args) executes the compiled NEFF on all 8 cores."""
    import jax
    import numpy as np
    from jax.experimental.shard_map import shard_map
    from jax.sharding import Mesh, NamedSharding, PartitionSpec

    from concourse import bass2jax, mybir
    from concourse.bass2jax import (
        _bass_exec_p, install_neuronx_cc_hook, partition_id_tensor,
    )

    install_neuronx_cc_hook()
    pname = nc.partition_id_tensor.name if nc.partition_id_tensor else None
    in_names, out_names, out_avals, zero_outs = [], [], [], []
    for alloc in nc.m.functions[0].allocations:
        if not isinstance(alloc, mybir.MemoryLocationSet):
            continue
        name = alloc.memorylocations[0].name
        if alloc.kind == "ExternalInput":
            if name == pname:
                continue
            in_names.append(name)
        elif alloc.kind == "ExternalOutput":
            shape = tuple(alloc.tensor_shape)
            dtype = mybir.dt.np(alloc.dtype)
            out_names.append(name)
            out_avals.append(jax.core.ShapedArray(shape, dtype))
            zero_outs.append(np.zeros(shape, dtype))
    n_params = len(in_names)
    all_names = in_names + out_names
    if pname is not None:
        all_names = all_names + [pname]

    chain_i = in_names.index("chain") if "chain" in in_names else None
    chain_o = out_names.index("chain_out") if "chain_out" in out_names else None

    def _body(*args):
        operands = list(args)
        if pname is not None:
            operands.append(partition_id_tensor())
        for _ in range(repeat):
            outs = _bass_exec_p.bind(
                *operands,
                out_avals=tuple(out_avals),
                in_names=tuple(all_names),
                out_names=tuple(out_names),
                lowering_input_output_aliases=(),
                sim_require_finite=True,
                sim_require_nnan=True,
                nc=nc,
            )
            if chain_i is not None:
                operands[chain_i] = outs[chain_o]
        return tuple(outs)

    devices = jax.devices()[:N_CORES]
    mesh = Mesh(np.asarray(devices), ("core",))
    spec = PartitionSpec("core")
    n_all = n_params + len(out_names)
    fn = jax.jit(
        shard_map(
            _body, mesh=mesh, in_specs=(spec,) * n_all,
            out_specs=(spec,) * len(out_names), check_rep=False,
        ),
        keep_unused=True,
    )

    def prep(in_maps):
        sharding = NamedSharding(mesh, spec)
        args = []
        for i, name in enumerate(in_names):
            cat = np.concatenate([np.asarray(m[name]) for m in in_maps], axis=0)
            args.append(jax.device_put(cat, sharding))
        for z in zero_outs:
            cat = np.zeros((N_CORES * z.shape[0], *z.shape[1:]), z.dtype)
            args.append(jax.device_put(cat, sharding))
        return args

    def unpack(outs):
        return {
            name: np.asarray(outs[i]).reshape(N_CORES, *out_avals[i].shape)[0]
            for i, name in enumerate(out_names)
        }

    return fn, prep, unpack


def kernel(x, edge_index, edge_attr, batch, W1, b1, W2, b2, Wo, bo, **_):
    per_core, plan, xt_virt = _pack_host(x, edge_index, edge_attr, batch)
    nc = _build_program(plan)

    common = dict(
        chain=np.zeros((1, 4), np.float32),
        xt=xt_virt,
        w1=np.asarray(W1, np.float32),
        w2=np.asarray(W2, np.float32),
        wo=np.asarray(Wo, np.float32),
        b1=np.tile(np.asarray(b1, np.float32).reshape(1, -1), (SLOTS, 1)),
        b2=np.tile(np.asarray(b2, np.float32).reshape(1, -1), (SLOTS, 1)),
        bo=np.tile(np.asarray(bo, np.float32).reshape(1, -1), (N_GRAPHS, 1)),
    )
    in_maps = []
    for c in range(N_CORES):
        m = dict(common)
        m.update(per_core[c])
        in_maps.append(m)

    from concourse.bass_utils import run_bass_kernel_spmd

    res = run_bass_kernel_spmd(nc, in_maps, list(range(N_CORES)))
    out = res.results[0]["out"]
    kernel.last_exec_time_ns = res.exec_time_ns
    kernel.last_results = res.results
    kernel.last_res = res
    return np.asarray(out, np.float32)


kernel.last_exec_time_ns = None



# revision 6
# speedup vs baseline: 1.7807x; 1.7807x over previous
"""Trainium2 Bass kernel for a 2-layer GCN (FCGraphGNN) over 8 NeuronCores.

Math (matches reference):
  ew' = [edge_attr; ones(N)]  (self loops), deg = segsum(ew', dst), dinv = deg^-1/2
  h1 = relu(segsum(dinv[src]*ew*dinv[dst] * (x@W1)[src]) + b1)
  h2 = relu(segsum(norm * (h1@W2)[src]) + b2)
  out = mean-pool-by-graph(h2) @ Wo + bo

Strategy (v2):
  - Shard edges by dst across 8 cores (sorted by dst host-side).
  - Virtual node space: 64-node windows, NW=128 windows/core, NV=65536.
  - Feature tables (h0 = dinv*(x@W1), h1glob = dinv*h1) stored bf16 as
    [NV, 128] rows (64 real features + 64 zero pad) so each dma_gather
    descriptor moves the minimum 256B.
  - Messages gathered per edge with dma_gather (int16 idx, lo/hi halves).
  - Segment-sum via PE: per 128-edge tile a host-PRECOMPUTED one-hot
    S[128,64] (bf16, entries=edge weight) is loaded from DRAM and
    matmul'd against the gathered messages into PSUM per window.
    (v1 built S on DVE per tile -> DVE was the bottleneck.)
  - dinv computed on device from a padded ewdeg table; dinv + h1
    all-gathered across cores; pooled partial sums all-reduced.
  - Pool one-hot Sg (graph ids) precomputed on host (junk bin absorbs
    unused slots, so no per-core count bookkeeping -> SPMD uniform).
"""

import os
import sys

import numpy as np

sys.path.insert(0, "/opt/trn_rl_repo")

# ---------------------------------------------------------------- constants
N_NODES = 50000
N_EDGES = 3200000
N_GRAPHS = 50
IN_F = 5
HID = 64
OUT_F = 2
N_CORES = 8

SLOTS = 64          # dst nodes per window
T_SIDE = 13         # 128-edge tiles per (window, src-half)
CAP = T_SIDE * 128  # edge slots per (window, side)
NW = 128            # windows per core (uniform across cores)
NVC = NW * SLOTS    # 8192 virtual rows per core
NV = N_CORES * NVC  # 65536
NVH = NV // 2       # 32768 (int16 idx budget per half)
GROUP_W = 4         # windows fetched per dma_gather pair
GI = GROUP_W * CAP  # idxs per gather call (6656)
IDXC = GI // 16
NG = NW // GROUP_W  # 32 groups
TPG = GROUP_W * T_SIDE  # 52 tiles per (group, side)
NT_SIDE = NW * T_SIDE   # 1664 tiles per side
SG = 52             # graph one-hot width (50 graphs + 2 junk bins)


def _pack_host(x, edge_index, edge_attr, batch):
    """Pure index/layout preprocessing (numpy). Returns per-core input dicts
    plus the static plan (DEG_K)."""
    import ml_dtypes

    bf16 = ml_dtypes.bfloat16

    src = np.asarray(edge_index[0], dtype=np.int64)
    dst = np.asarray(edge_index[1], dtype=np.int64)
    ew = np.asarray(edge_attr, dtype=np.float32).reshape(-1)
    loop = np.arange(N_NODES, dtype=np.int64)
    src = np.concatenate([src, loop]).astype(np.int32)
    dst = np.concatenate([dst, loop]).astype(np.int32)
    ew = np.concatenate([ew, np.ones(N_NODES, np.float32)])
    E = src.shape[0]

    deg_cnt = np.bincount(dst, minlength=N_NODES).astype(np.int64)
    node_ptr = np.zeros(N_NODES + 1, np.int64)
    np.cumsum(deg_cnt, out=node_ptr[1:])
    order = np.argsort(dst, kind="stable")

    # core node boundaries balancing edge counts
    cum = node_ptr[1:]
    nb = [0]
    for c in range(1, N_CORES):
        nb.append(int(np.searchsorted(cum, c * E / N_CORES)))
    nb.append(N_NODES)
    nb = np.array(nb, np.int64)
    split_node = int(nb[4])  # src < split_node -> "lo" half of virtual space

    side_lo = src < split_node
    deg_lo = np.bincount(dst[side_lo], minlength=N_NODES).astype(np.int64)
    deg_hi = deg_cnt - deg_lo

    # window packing per core: <=64 nodes AND <=CAP edges per side
    core_windows = []
    for c in range(N_CORES):
        wlist = []
        v = int(nb[c])
        end = int(nb[c + 1])
        while v < end:
            ws = v
            lo = hi = cnt = 0
            while (
                v < end
                and cnt < SLOTS
                and lo + deg_lo[v] <= CAP
                and hi + deg_hi[v] <= CAP
            ):
                lo += int(deg_lo[v])
                hi += int(deg_hi[v])
                cnt += 1
                v += 1
            wlist.append((ws, v))
        assert len(wlist) <= NW, f"core {c}: {len(wlist)} windows > NW={NW}"
        core_windows.append(wlist)

    # vid map (node -> virtual id)
    node_vid = np.zeros(N_NODES, np.int32)
    for c in range(N_CORES):
        for w, (ws, we) in enumerate(core_windows[c]):
            node_vid[ws:we] = c * NVC + w * SLOTS + np.arange(we - ws, dtype=np.int32)

    DEG_K = int(deg_cnt.max())
    DEG_K = (DEG_K + 3) // 4 * 4

    # per-side dst-sorted edge lists + ptrs
    lo_edges = order[side_lo[order]]
    hi_edges = order[~side_lo[order]]
    lo_ptr = np.zeros(N_NODES + 1, np.int64)
    np.cumsum(deg_lo, out=lo_ptr[1:])
    hi_ptr = np.zeros(N_NODES + 1, np.int64)
    np.cumsum(deg_hi, out=hi_ptr[1:])

    vid_src = node_vid[src]

    # global position of each edge within its dst node's sorted run
    col_within = np.empty(E, np.int64)
    ar = np.arange(E, dtype=np.int64)
    col_within[order] = ar - node_ptr[dst[order]]

    # xt in virtual layout (shared by all cores), bf16
    xt_virt = np.zeros((IN_F, NV), np.float32)
    xt_virt[:, node_vid] = np.asarray(x, np.float32).T
    xt_virt = xt_virt.astype(bf16)

    batch_i = np.asarray(batch, np.int64)

    per_core = []
    for c in range(N_CORES):
        wlist = core_windows[c]
        idx_lo = np.zeros((NW, CAP), np.int16)
        idx_hi = np.zeros((NW, CAP), np.int16)
        # S tiles: [side][tile, row, slot] with tile = w*T_SIDE + ti
        s_lo = np.zeros((NT_SIDE, 128, SLOTS), np.float32)
        s_hi = np.zeros((NT_SIDE, 128, SLOTS), np.float32)
        gid = np.full((SLOTS, NW), 50.0, np.float32)
        ewdeg = np.zeros((NVC, DEG_K), np.float32)

        for w, (ws, we) in enumerate(wlist):
            for edges, ptr, idxbuf, sbuf, voff in (
                (lo_edges, lo_ptr, idx_lo, s_lo, 0),
                (hi_edges, hi_ptr, idx_hi, s_hi, NVH),
            ):
                ids = edges[ptr[ws] : ptr[we]]
                n = ids.shape[0]
                idxbuf[w, :n] = (vid_src[ids] - voff).astype(np.int16)
                slot_in = np.arange(n, dtype=np.int64)
                t = w * T_SIDE + slot_in // 128
                r = slot_in % 128
                sbuf[t, r, (dst[ids] - ws)] = ew[ids]
            gid[: we - ws, w] = batch_i[ws:we].astype(np.float32)

        # padded per-node edge weights for the degree pass
        e_lo = int(node_ptr[nb[c]])
        e_hi = int(node_ptr[nb[c + 1]])
        es = order[e_lo:e_hi]
        rows = node_vid[dst[es]] - c * NVC
        ewdeg[rows, col_within[es]] = ew[es]
        rowdeg = np.zeros(NVC, np.int64)
        nr = node_vid[nb[c] : nb[c + 1]] - c * NVC
        rowdeg[nr] = deg_cnt[nb[c] : nb[c + 1]]
        ewdeg[rowdeg == 0, 0] = 1.0

        # pool one-hot: [SLOTS, NW, SG] -> flat [SLOTS, NW*SG] bf16
        sgp = np.zeros((SLOTS, NW, SG), np.float32)
        ss, ww = np.meshgrid(np.arange(SLOTS), np.arange(NW), indexing="ij")
        sgp[ss, ww, gid.astype(np.int64)] = 1.0
        sgp = sgp.reshape(SLOTS, NW * SG).astype(bf16)

        # wrap gather indices: [NG, 128, IDXC] (16-partition wrap, replicated)
        def wrap(a):
            g = a.reshape(NG, GI // 16, 16).transpose(0, 2, 1)  # [NG,16,IDXC]
            return np.ascontiguousarray(np.tile(g, (1, 8, 1)))

        # S tiles in DMA layout: [128, NT_SIDE*SLOTS] bf16
        def slayout(s):
            return np.ascontiguousarray(
                s.transpose(1, 0, 2).reshape(128, NT_SIDE * SLOTS).astype(bf16)
            )

        per_core.append(
            dict(
                slo=slayout(s_lo),
                shi=slayout(s_hi),
                idxlo=wrap(idx_lo.reshape(-1)),
                idxhi=wrap(idx_hi.reshape(-1)),
                sgp=sgp,
                ewdeg=ewdeg,
            )
        )

    plan = dict(DEG_K=DEG_K)
    return per_core, plan, xt_virt


def _build_program(plan):
    import concourse.bacc as bacc
    import concourse.bass as bass
    import concourse.tile as tile
    from concourse import mybir
    from concourse.tile_rust import add_dep_helper

    f32 = mybir.dt.float32
    bf16 = mybir.dt.bfloat16
    i16 = mybir.dt.int16
    Alu = mybir.AluOpType
    Act = mybir.ActivationFunctionType

    DEG_K = plan["DEG_K"]

    NQ = int(os.environ.get("KNQ", "4"))
    nc = bacc.Bacc("TRN2", target_bir_lowering=False, debug=False,
                   num_devices=N_CORES, num_swdge_queues=NQ)

    xt = nc.declare_dram_parameter("xt", [IN_F, NV], bf16, isOutput=False)
    w1 = nc.declare_dram_parameter("w1", [IN_F, HID], bf16, isOutput=False)
    w2 = nc.declare_dram_parameter("w2", [HID, HID], bf16, isOutput=False)
    wo = nc.declare_dram_parameter("wo", [HID, OUT_F], f32, isOutput=False)
    b1 = nc.declare_dram_parameter("b1", [SLOTS, HID], f32, isOutput=False)
    b2 = nc.declare_dram_parameter("b2", [SLOTS, HID], f32, isOutput=False)
    bo = nc.declare_dram_parameter("bo", [N_GRAPHS, OUT_F], f32, isOutput=False)
    ewdeg = nc.declare_dram_parameter("ewdeg", [NVC, DEG_K], f32, isOutput=False)
    slo_d = nc.declare_dram_parameter("slo", [128, NT_SIDE * SLOTS], bf16,
                                      isOutput=False)
    shi_d = nc.declare_dram_parameter("shi", [128, NT_SIDE * SLOTS], bf16,
                                      isOutput=False)
    idxlo = nc.declare_dram_parameter("idxlo", [NG, 128, IDXC], i16, isOutput=False)
    idxhi = nc.declare_dram_parameter("idxhi", [NG, 128, IDXC], i16, isOutput=False)
    sgp_d = nc.declare_dram_parameter("sgp", [SLOTS, NW * SG], bf16, isOutput=False)
    out = nc.declare_dram_parameter("out", [N_GRAPHS, OUT_F], f32, isOutput=True)
    chain_in = nc.declare_dram_parameter("chain", [1, 4], f32, isOutput=False)
    chain_out = nc.declare_dram_parameter("chain_out", [1, 4], f32, isOutput=True)

    groups = [list(range(N_CORES))]

    with tile.TileContext(nc) as tc:
        with (
            tc.tile_pool(name="dram", bufs=1, space="DRAM") as dram,
            tc.tile_pool(name="const", bufs=1) as cpool,
            tc.tile_pool(name="persist", bufs=1) as ppool,
        ):
            h0 = dram.tile([NV, 128], bf16, tag="h0")
            h1loc = dram.tile([NVC, 128], bf16, tag="h1loc")
            h1glob = dram.tile([NV, 128], bf16, tag="h1glob")
            dinv_loc_d = dram.tile([NVC], f32, tag="dinvloc")
            dinv_glob_d = dram.tile([N_CORES, NVC], f32, tag="dinvglob")
            pool_in_d = dram.tile([HID + 1, SG], f32, tag="poolin")
            pool_out_d = dram.tile([HID + 1, SG], f32, tag="poolout")

            # ---- constants
            w1s = cpool.tile([IN_F, HID], bf16, tag="w1s")
            nc.sync.dma_start(w1s[:], w1[:])
            w2s = cpool.tile([HID, HID], bf16, tag="w2s")
            nc.sync.dma_start(w2s[:], w2[:])
            wos = cpool.tile([HID, OUT_F], f32, tag="wos")
            nc.sync.dma_start(wos[:], wo[:])
            b1s = cpool.tile([SLOTS, HID], f32, tag="b1s")
            nc.sync.dma_start(b1s[:], b1[:])
            b2s = cpool.tile([SLOTS, HID], f32, tag="b2s")
            nc.sync.dma_start(b2s[:], b2[:])
            bos = cpool.tile([N_GRAPHS, OUT_F], f32, tag="bos")
            nc.sync.dma_start(bos[:], bo[:])
            sgs = cpool.tile([SLOTS, NW * SG], bf16, tag="sgs")
            nc.sync.dma_start(sgs[:], sgp_d[:])

            dinvw = ppool.tile([SLOTS, NW], f32, tag="dinvw")
            d128 = ppool.tile([128, N_CORES, NW // 2], f32, tag="d128")

            # ---- degree pass -> local dinv (window-slot layout)
            with tc.tile_pool(name="deg", bufs=1) as dpool:
                degt = dpool.tile([SLOTS, NW, DEG_K], f32, tag="degt")
                nc.sync.dma_start(
                    degt[:], ewdeg[:].rearrange("(w s) k -> s w k", s=SLOTS)
                )
                deg = dpool.tile([SLOTS, NW], f32, tag="deg")
                nc.vector.tensor_reduce(
                    out=deg[:], in_=degt[:], axis=mybir.AxisListType.X,
                    op=Alu.add,
                )
                rec = dpool.tile([SLOTS, NW], f32, tag="rec")
                nc.vector.reciprocal(rec[:], deg[:])
                nc.scalar.activation(dinvw[:], rec[:], Act.Sqrt)
                nc.sync.dma_start(
                    dinv_loc_d[:].rearrange("(w s) -> s w", s=SLOTS), dinvw[:]
                )

            # all-gather dinv; load in [128, r, NW/2] chunk layout for preamble
            nc.gpsimd.collective_compute(
                "AllGather", Alu.bypass, replica_groups=groups,
                ins=[dinv_loc_d[:]],
                outs=[dinv_glob_d[:].rearrange("r n -> (r n)")],
            )
            nc.sync.dma_start(
                d128[:],
                dinv_glob_d[:].rearrange("r (c p) -> p r c", p=128),
            )

            # ---- preamble: h0 = dinv * (x @ W1) in bf16 [NV, 128] rows
            PB = 8  # 128-row chunks per h0 write
            h0_writes = []
            with (
                tc.tile_pool(name="pre", bufs=2) as prepool,
                tc.tile_pool(name="prepsum", bufs=4, space="PSUM") as pspool,
                tc.tile_pool(name="prestage", bufs=2) as stpool,
            ):
                for r in range(N_CORES):
                    xts = prepool.tile([IN_F, NVC], bf16, tag="xts")
                    nc.sync.dma_start(xts[:], xt[:, r * NVC : (r + 1) * NVC])
                    for cb in range(NW // 2 // PB):
                        stage = stpool.tile([128, PB, 128], bf16, tag="h0st")
                        nc.vector.memset(stage[:, :, HID:128], 0.0)
                        for j in range(PB):
                            ch = cb * PB + j
                            ps = pspool.tile([128, HID], f32, tag="prepsum")
                            nc.tensor.matmul(
                                out=ps[:],
                                lhsT=xts[:, ch * 128 : (ch + 1) * 128],
                                rhs=w1s[:],
                                start=True, stop=True,
                            )
                            nc.scalar.activation(
                                stage[:, j, 0:HID], ps[:], Act.Copy,
                                scale=d128[:, r, ch : ch + 1],
                            )
                        row0 = r * NVC + cb * PB * 128
                        h0_writes.append(
                            nc.sync.dma_start(
                                h0[row0 : row0 + PB * 128, :].rearrange(
                                    "(b p) h -> p b h", p=128
                                ),
                                stage[:],
                            )
                        )

            fence0 = nc.gpsimd.engine_nop()
            for wi in h0_writes:
                add_dep_helper(fence0.ins, wi.ins,
                               reason="h0 table ready before gathers")

            # ---- message-passing layer (one pass over the edge tiles)
            def layer(l, src_table, fence, pool_ps=None):
                lo_view = src_table[0:NVH, :]
                hi_view = src_table[NVH:NV, :]
                with (
                    tc.tile_pool(name=f"idx{l}", bufs=4) as ipool,
                    tc.tile_pool(name=f"mbuf{l}", bufs=2) as mpool,
                    tc.tile_pool(name=f"sbuf{l}", bufs=2) as spool,
                    tc.tile_pool(name=f"wpsum{l}", bufs=4, space="PSUM") as wpool,
                    tc.tile_pool(name=f"epi{l}", bufs=3) as epool,
                    tc.tile_pool(name=f"p2_{l}", bufs=2, space="PSUM") as p2pool,
                ):
                    for g in range(NG):
                        ilo = ipool.tile([128, IDXC], i16, tag="ilo")
                        nc.sync.dma_start(ilo[:], idxlo[g])
                        ihi = ipool.tile([128, IDXC], i16, tag="ihi")
                        nc.sync.dma_start(ihi[:], idxhi[g])
                        mlo = mpool.tile([128, TPG, 128], bf16, tag="mlo")
                        mhi = mpool.tile([128, TPG, 128], bf16, tag="mhi")
                        glo = nc.gpsimd.dma_gather(
                            mlo[:], lo_view, ilo[:], GI, GI, 128,
                            single_packet=False,
                            queue_num=(2 * g) % NQ,
                        )
                        ghi = nc.gpsimd.dma_gather(
                            mhi[:], hi_view, ihi[:], GI, GI, 128,
                            single_packet=False,
                            queue_num=(2 * g + 1) % NQ,
                        )
                        if fence is not None:
                            add_dep_helper(glo.ins, fence.ins,
                                           reason="gather src table ready")
                            add_dep_helper(ghi.ins, fence.ins,
                                           reason="gather src table ready")
                        slo = spool.tile([128, TPG * SLOTS], bf16, tag="slo")
                        nc.sync.dma_start(
                            slo[:], slo_d[:, g * TPG * SLOTS : (g + 1) * TPG * SLOTS]
                        )
                        shi = spool.tile([128, TPG * SLOTS], bf16, tag="shi")
                        nc.sync.dma_start(
                            shi[:], shi_d[:, g * TPG * SLOTS : (g + 1) * TPG * SLOTS]
                        )
                        for wl in range(GROUP_W):
                            w = g * GROUP_W + wl
                            ps = wpool.tile([SLOTS, HID], f32, tag="wps")
                            k = 0
                            for mb, sb in ((mlo, slo), (mhi, shi)):
                                for ti in range(T_SIDE):
                                    blk = wl * T_SIDE + ti
                                    stile = sb[:, blk * SLOTS : (blk + 1) * SLOTS]
                                    first = k == 0
                                    last = k == 2 * T_SIDE - 1
                                    if l == 1:
                                        nc.tensor.matmul(
                                            out=ps[:], lhsT=stile,
                                            rhs=mb[:, blk, 0:HID],
                                            start=first, stop=last,
                                        )
                                    else:
                                        nc.tensor.matmul(
                                            out=ps[:], lhsT=mb[:, blk, 0:HID],
                                            rhs=stile,
                                            start=first, stop=last,
                                        )
                                    k += 1
                            dv = dinvw[:, w : w + 1]
                            if l == 1:
                                u = epool.tile([SLOTS, HID], f32, tag="u1")
                                nc.vector.tensor_scalar(
                                    out=u[:], in0=ps[:], scalar1=dv,
                                    scalar2=None, op0=Alu.mult,
                                )
                                nc.vector.tensor_tensor(
                                    out=u[:], in0=u[:], in1=b1s[:], op=Alu.add,
                                )
                                uh = epool.tile([SLOTS, 128], bf16, tag="uh")
                                nc.vector.memset(uh[:, HID:128], 0.0)
                                nc.vector.tensor_scalar(
                                    out=u[:], in0=u[:], scalar1=0.0,
                                    scalar2=None, op0=Alu.max,
                                )
                                nc.vector.tensor_scalar(
                                    out=uh[:, 0:HID], in0=u[:], scalar1=dv,
                                    scalar2=None, op0=Alu.mult,
                                )
                                nc.sync.dma_start(
                                    h1loc[w * SLOTS : (w + 1) * SLOTS, :],
                                    uh[:],
                                )
                            else:
                                aggT = epool.tile([HID, SLOTS], bf16, tag="aggT")
                                nc.vector.tensor_copy(aggT[:], ps[:])
                                ps2 = p2pool.tile([SLOTS, HID], f32, tag="ps2")
                                nc.tensor.matmul(
                                    out=ps2[:], lhsT=aggT[:], rhs=w2s[:],
                                    start=True, stop=True,
                                )
                                u = epool.tile([SLOTS, HID + 1], bf16, tag="u2")
                                nc.vector.memset(u[:, HID : HID + 1], 1.0)
                                uf = epool.tile([SLOTS, HID], f32, tag="u2f")
                                nc.vector.tensor_scalar(
                                    out=uf[:], in0=ps2[:], scalar1=dv,
                                    scalar2=None, op0=Alu.mult,
                                )
                                nc.vector.tensor_tensor(
                                    out=uf[:], in0=uf[:], in1=b2s[:], op=Alu.add,
                                )
                                nc.vector.tensor_scalar(
                                    out=u[:, 0:HID], in0=uf[:], scalar1=0.0,
                                    scalar2=None, op0=Alu.max,
                                )
                                nc.tensor.matmul(
                                    out=pool_ps[:], lhsT=u[:],
                                    rhs=sgs[:, w * SG : (w + 1) * SG],
                                    start=(w == 0),
                                    stop=(w == NW - 1),
                                )
                    if l == 2:
                        pst = epool.tile([HID + 1, SG], f32, tag="pst")
                        nc.vector.tensor_copy(pst[:], pool_ps[:])
                        nc.sync.dma_start(pool_in_d[:], pst[:])

            layer(1, h0, fence0)

            cc_h1 = nc.gpsimd.collective_compute(
                "AllGather", Alu.bypass, replica_groups=groups,
                ins=[h1loc[:].rearrange("a b -> (a b)")],
                outs=[h1glob[:].rearrange("a b -> (a b)")],
            )

            with tc.tile_pool(name="gps", bufs=1, space="PSUM") as gpool:
                pool_ps = gpool.tile([HID + 1, SG], f32, tag="poolps")
                layer(2, h1glob, cc_h1, pool_ps=pool_ps)

            # ---- pooled partial sums -> all-reduce -> final linear
            nc.gpsimd.collective_compute(
                "AllReduce", Alu.add, replica_groups=groups,
                ins=[pool_in_d[:]], outs=[pool_out_d[:]],
            )
            with (
                tc.tile_pool(name="fin", bufs=1) as fpool,
                tc.tile_pool(name="finps", bufs=1, space="PSUM") as fpsum,
            ):
                pr = fpool.tile([HID + 1, SG], f32, tag="pr")
                nc.sync.dma_start(pr[:], pool_out_d[:])
                cm = fpool.tile([1, SG], f32, tag="cm")
                nc.vector.tensor_scalar(
                    out=cm[:], in0=pr[HID : HID + 1, :], scalar1=1.0,
                    scalar2=None, op0=Alu.max,
                )
                rcp = fpool.tile([1, SG], f32, tag="rcp")
                nc.vector.reciprocal(rcp[:], cm[:])
                rcpb = fpool.tile([HID, SG], f32, tag="rcpb")
                nc.gpsimd.partition_broadcast(rcpb[:], rcp[:])
                pooledT = fpool.tile([HID, N_GRAPHS], f32, tag="pooledT")
                nc.vector.tensor_tensor(
                    out=pooledT[:], in0=pr[0:HID, 0:N_GRAPHS],
                    in1=rcpb[0:HID, 0:N_GRAPHS],
                    op=Alu.mult,
                )
                pso = fpsum.tile([N_GRAPHS, OUT_F], f32, tag="pso")
                nc.tensor.matmul(
                    out=pso[:], lhsT=pooledT[:], rhs=wos[:],
                    start=True, stop=True,
                )
                osb = fpool.tile([N_GRAPHS, OUT_F], f32, tag="osb")
                nc.vector.tensor_tensor(
                    out=osb[:], in0=pso[:],
                    in1=bos[:],
                    op=Alu.add,
                )
                nc.sync.dma_start(out[:], osb[:])
                chs = fpool.tile([1, 4], f32, tag="chs")
                nc.sync.dma_start(chs[:], chain_in[:])
                nc.vector.tensor_scalar_add(chs[:], chs[:], 1.0)
                nc.sync.dma_start(chain_out[:], chs[:])

    nc.compile()
    return nc


def kernel(x, edge_index, edge_attr, batch, W1, b1, W2, b2, Wo, bo, **_):
    import ml_dtypes

    bf16 = ml_dtypes.bfloat16

    per_core, plan, xt_virt = _pack_host(x, edge_index, edge_attr, batch)
    nc = _build_program(plan)

    common = dict(
        chain=np.zeros((1, 4), np.float32),
        xt=xt_virt,
        w1=np.asarray(W1, np.float32).astype(bf16),
        w2=np.asarray(W2, np.float32).astype(bf16),
        wo=np.asarray(Wo, np.float32),
        b1=np.tile(np.asarray(b1, np.float32).reshape(1, -1), (SLOTS, 1)),
        b2=np.tile(np.asarray(b2, np.float32).reshape(1, -1), (SLOTS, 1)),
        bo=np.tile(np.asarray(bo, np.float32).reshape(1, -1), (N_GRAPHS, 1)),
    )
    in_maps = []
    for c in range(N_CORES):
        m = dict(common)
        m.update(per_core[c])
        in_maps.append(m)

    from concourse.bass_utils import run_bass_kernel_spmd

    res = run_bass_kernel_spmd(nc, in_maps, list(range(N_CORES)))
    out = res.results[0]["out"]
    kernel.last_exec_time_ns = res.exec_time_ns
    kernel.last_results = res.results
    kernel.last_res = res
    return np.asarray(out, np.float32)


kernel.last_exec_time_ns = None


# revision 7
# speedup vs baseline: 2.6903x; 1.5108x over previous
"""Trainium2 Bass kernel for a 2-layer GCN (FCGraphGNN) over 8 NeuronCores.

Math (matches reference):
  ew' = [edge_attr; ones(N)]  (self loops), deg = segsum(ew', dst), dinv = deg^-1/2
  h1 = relu(segsum(dinv[src]*ew*dinv[dst] * (x@W1)[src]) + b1)
  h2 = relu(segsum(norm * (h1@W2)[src]) + b2)
  out = mean-pool-by-graph(h2) @ Wo + bo

Strategy (v2):
  - Shard edges by dst across 8 cores (sorted by dst host-side).
  - Virtual node space: 64-node windows, NW=128 windows/core, NV=65536.
  - Feature tables (h0 = dinv*(x@W1), h1glob = dinv*h1) stored bf16 as
    [NV, 128] rows (64 real features + 64 zero pad) so each dma_gather
    descriptor moves the minimum 256B.
  - Messages gathered per edge with dma_gather (int16 idx, lo/hi halves).
  - Segment-sum via PE: per 128-edge tile a host-PRECOMPUTED one-hot
    S[128,64] (bf16, entries=edge weight) is loaded from DRAM and
    matmul'd against the gathered messages into PSUM per window.
    (v1 built S on DVE per tile -> DVE was the bottleneck.)
  - dinv computed on device from a padded ewdeg table; dinv + h1
    all-gathered across cores; pooled partial sums all-reduced.
  - Pool one-hot Sg (graph ids) precomputed on host (junk bin absorbs
    unused slots, so no per-core count bookkeeping -> SPMD uniform).
"""

import os
import sys

import numpy as np

sys.path.insert(0, "/opt/trn_rl_repo")

# ---------------------------------------------------------------- constants
N_NODES = 50000
N_EDGES = 3200000
N_GRAPHS = 50
IN_F = 5
HID = 64
OUT_F = 2
N_CORES = 8

SLOTS = 64          # dst nodes per window
T_SIDE = 13         # 128-edge tiles per (window, src-half)
CAP = T_SIDE * 128  # edge slots per (window, side)
NW = 128            # windows per core (uniform across cores)
NVC = NW * SLOTS    # 8192 virtual rows per core
NV = N_CORES * NVC  # 65536
NVH = NV // 2       # 32768 (int16 idx budget per half)
GROUP_W = 4         # windows fetched per dma_gather pair
GI = GROUP_W * CAP  # idxs per gather call (6656)
IDXC = GI // 16
NG = NW // GROUP_W  # 32 groups
TPG = GROUP_W * T_SIDE  # 52 tiles per (group, side)
NT_SIDE = NW * T_SIDE   # 1664 tiles per side
SG = 52             # graph one-hot width (50 graphs + 2 junk bins)


def _pack_host(x, edge_index, edge_attr, batch):
    """Pure index/layout preprocessing (numpy). Returns per-core input dicts
    plus the static plan (DEG_K)."""
    import ml_dtypes

    bf16 = ml_dtypes.bfloat16

    src = np.asarray(edge_index[0], dtype=np.int64)
    dst = np.asarray(edge_index[1], dtype=np.int64)
    ew = np.asarray(edge_attr, dtype=np.float32).reshape(-1)
    loop = np.arange(N_NODES, dtype=np.int64)
    src = np.concatenate([src, loop]).astype(np.int32)
    dst = np.concatenate([dst, loop]).astype(np.int32)
    ew = np.concatenate([ew, np.ones(N_NODES, np.float32)])
    E = src.shape[0]

    deg_cnt = np.bincount(dst, minlength=N_NODES).astype(np.int64)
    node_ptr = np.zeros(N_NODES + 1, np.int64)
    np.cumsum(deg_cnt, out=node_ptr[1:])
    order = np.argsort(dst, kind="stable")

    # core node boundaries balancing edge counts
    cum = node_ptr[1:]
    nb = [0]
    for c in range(1, N_CORES):
        nb.append(int(np.searchsorted(cum, c * E / N_CORES)))
    nb.append(N_NODES)
    nb = np.array(nb, np.int64)
    split_node = int(nb[4])  # src < split_node -> "lo" half of virtual space

    side_lo = src < split_node
    deg_lo = np.bincount(dst[side_lo], minlength=N_NODES).astype(np.int64)
    deg_hi = deg_cnt - deg_lo

    # window packing per core: <=64 nodes AND <=CAP edges per side
    core_windows = []
    for c in range(N_CORES):
        wlist = []
        v = int(nb[c])
        end = int(nb[c + 1])
        while v < end:
            ws = v
            lo = hi = cnt = 0
            while (
                v < end
                and cnt < SLOTS
                and lo + deg_lo[v] <= CAP
                and hi + deg_hi[v] <= CAP
            ):
                lo += int(deg_lo[v])
                hi += int(deg_hi[v])
                cnt += 1
                v += 1
            wlist.append((ws, v))
        assert len(wlist) <= NW, f"core {c}: {len(wlist)} windows > NW={NW}"
        core_windows.append(wlist)

    # vid map (node -> virtual id)
    node_vid = np.zeros(N_NODES, np.int32)
    for c in range(N_CORES):
        for w, (ws, we) in enumerate(core_windows[c]):
            node_vid[ws:we] = c * NVC + w * SLOTS + np.arange(we - ws, dtype=np.int32)

    DEG_K = int(deg_cnt.max())
    DEG_K = (DEG_K + 3) // 4 * 4

    # per-side dst-sorted edge lists + ptrs
    lo_edges = order[side_lo[order]]
    hi_edges = order[~side_lo[order]]
    lo_ptr = np.zeros(N_NODES + 1, np.int64)
    np.cumsum(deg_lo, out=lo_ptr[1:])
    hi_ptr = np.zeros(N_NODES + 1, np.int64)
    np.cumsum(deg_hi, out=hi_ptr[1:])

    vid_src = node_vid[src]

    # global position of each edge within its dst node's sorted run
    col_within = np.empty(E, np.int64)
    ar = np.arange(E, dtype=np.int64)
    col_within[order] = ar - node_ptr[dst[order]]

    # xt in virtual layout (shared by all cores), bf16
    xt_virt = np.zeros((IN_F, NV), np.float32)
    xt_virt[:, node_vid] = np.asarray(x, np.float32).T
    xt_virt = xt_virt.astype(bf16)

    batch_i = np.asarray(batch, np.int64)

    per_core = []
    for c in range(N_CORES):
        wlist = core_windows[c]
        idx_lo = np.zeros((NW, CAP), np.int16)
        idx_hi = np.zeros((NW, CAP), np.int16)
        # S tiles: [side][tile, row, slot] with tile = w*T_SIDE + ti
        s_lo = np.zeros((NT_SIDE, 128, SLOTS), np.float32)
        s_hi = np.zeros((NT_SIDE, 128, SLOTS), np.float32)
        gid = np.full((SLOTS, NW), 50.0, np.float32)
        ewdeg = np.zeros((NVC, DEG_K), np.float32)

        for w, (ws, we) in enumerate(wlist):
            for edges, ptr, idxbuf, sbuf, voff in (
                (lo_edges, lo_ptr, idx_lo, s_lo, 0),
                (hi_edges, hi_ptr, idx_hi, s_hi, NVH),
            ):
                ids = edges[ptr[ws] : ptr[we]]
                n = ids.shape[0]
                idxbuf[w, :n] = (vid_src[ids] - voff).astype(np.int16)
                slot_in = np.arange(n, dtype=np.int64)
                t = w * T_SIDE + slot_in // 128
                r = slot_in % 128
                sbuf[t, r, (dst[ids] - ws)] = ew[ids]
            gid[: we - ws, w] = batch_i[ws:we].astype(np.float32)

        # padded per-node edge weights for the degree pass
        e_lo = int(node_ptr[nb[c]])
        e_hi = int(node_ptr[nb[c + 1]])
        es = order[e_lo:e_hi]
        rows = node_vid[dst[es]] - c * NVC
        ewdeg[rows, col_within[es]] = ew[es]
        rowdeg = np.zeros(NVC, np.int64)
        nr = node_vid[nb[c] : nb[c + 1]] - c * NVC
        rowdeg[nr] = deg_cnt[nb[c] : nb[c + 1]]
        ewdeg[rowdeg == 0, 0] = 1.0

        # pool one-hot: [SLOTS, NW, SG] -> flat [SLOTS, NW*SG] bf16
        sgp = np.zeros((SLOTS, NW, SG), np.float32)
        ss, ww = np.meshgrid(np.arange(SLOTS), np.arange(NW), indexing="ij")
        sgp[ss, ww, gid.astype(np.int64)] = 1.0
        sgp = sgp.reshape(SLOTS, NW * SG).astype(bf16)

        # wrap gather indices: [NG, 128, IDXC] (16-partition wrap, replicated)
        def wrap(a):
            g = a.reshape(NG, GI // 16, 16).transpose(0, 2, 1)  # [NG,16,IDXC]
            return np.ascontiguousarray(np.tile(g, (1, 8, 1)))

        # S tiles in DMA layout: [128, NT_SIDE*SLOTS] bf16
        def slayout(s):
            return np.ascontiguousarray(
                s.transpose(1, 0, 2).reshape(128, NT_SIDE * SLOTS).astype(bf16)
            )

        per_core.append(
            dict(
                slo=slayout(s_lo),
                shi=slayout(s_hi),
                idxlo=wrap(idx_lo.reshape(-1)),
                idxhi=wrap(idx_hi.reshape(-1)),
                sgp=sgp,
                ewdeg=ewdeg,
            )
        )

    plan = dict(DEG_K=DEG_K)
    return per_core, plan, xt_virt


def _build_program(plan):
    import concourse.bacc as bacc
    import concourse.bass as bass
    import concourse.tile as tile
    from concourse import mybir
    from concourse.tile_rust import add_dep_helper

    f32 = mybir.dt.float32
    bf16 = mybir.dt.bfloat16
    i16 = mybir.dt.int16
    Alu = mybir.AluOpType
    Act = mybir.ActivationFunctionType

    DEG_K = plan["DEG_K"]

    NQ = int(os.environ.get("KNQ", "4"))
    nc = bacc.Bacc("TRN2", target_bir_lowering=False, debug=False,
                   num_devices=N_CORES, num_swdge_queues=NQ)

    xt = nc.declare_dram_parameter("xt", [IN_F, NV], bf16, isOutput=False)
    w1 = nc.declare_dram_parameter("w1", [IN_F, HID], bf16, isOutput=False)
    w2 = nc.declare_dram_parameter("w2", [HID, HID], bf16, isOutput=False)
    wo = nc.declare_dram_parameter("wo", [HID, OUT_F], f32, isOutput=False)
    b1 = nc.declare_dram_parameter("b1", [SLOTS, HID], f32, isOutput=False)
    b2 = nc.declare_dram_parameter("b2", [SLOTS, HID], f32, isOutput=False)
    bo = nc.declare_dram_parameter("bo", [N_GRAPHS, OUT_F], f32, isOutput=False)
    ewdeg = nc.declare_dram_parameter("ewdeg", [NVC, DEG_K], f32, isOutput=False)
    slo_d = nc.declare_dram_parameter("slo", [128, NT_SIDE * SLOTS], bf16,
                                      isOutput=False)
    shi_d = nc.declare_dram_parameter("shi", [128, NT_SIDE * SLOTS], bf16,
                                      isOutput=False)
    idxlo = nc.declare_dram_parameter("idxlo", [NG, 128, IDXC], i16, isOutput=False)
    idxhi = nc.declare_dram_parameter("idxhi", [NG, 128, IDXC], i16, isOutput=False)
    sgp_d = nc.declare_dram_parameter("sgp", [SLOTS, NW * SG], bf16, isOutput=False)
    out = nc.declare_dram_parameter("out", [N_GRAPHS, OUT_F], f32, isOutput=True)
    chain_in = nc.declare_dram_parameter("chain", [1, 4], f32, isOutput=False)
    chain_out = nc.declare_dram_parameter("chain_out", [1, 4], f32, isOutput=True)

    groups = [list(range(N_CORES))]

    with tile.TileContext(nc) as tc:
        with (
            tc.tile_pool(name="dram", bufs=1, space="DRAM") as dram,
            tc.tile_pool(name="const", bufs=1) as cpool,
            tc.tile_pool(name="persist", bufs=1) as ppool,
        ):
            h0 = dram.tile([NV, 128], bf16, tag="h0")
            h1loc = dram.tile([NVC, 128], bf16, tag="h1loc")
            h1glob = dram.tile([NV, 128], bf16, tag="h1glob")
            dinv_loc_d = dram.tile([NVC], f32, tag="dinvloc")
            dinv_glob_d = dram.tile([N_CORES, NVC], f32, tag="dinvglob")
            pool_in_d = dram.tile([HID + 1, SG], f32, tag="poolin")
            pool_out_d = dram.tile([HID + 1, SG], f32, tag="poolout")

            # ---- constants
            w1s = cpool.tile([IN_F, HID], bf16, tag="w1s")
            nc.sync.dma_start(w1s[:], w1[:])
            w2s = cpool.tile([HID, HID], bf16, tag="w2s")
            nc.sync.dma_start(w2s[:], w2[:])
            wos = cpool.tile([HID, OUT_F], f32, tag="wos")
            nc.sync.dma_start(wos[:], wo[:])
            b1s = cpool.tile([SLOTS, HID], f32, tag="b1s")
            nc.sync.dma_start(b1s[:], b1[:])
            b2s = cpool.tile([SLOTS, HID], f32, tag="b2s")
            nc.sync.dma_start(b2s[:], b2[:])
            bos = cpool.tile([N_GRAPHS, OUT_F], f32, tag="bos")
            nc.sync.dma_start(bos[:], bo[:])
            sgs = cpool.tile([SLOTS, NW * SG], bf16, tag="sgs")
            nc.sync.dma_start(sgs[:], sgp_d[:])

            dinvw = ppool.tile([SLOTS, NW], f32, tag="dinvw")
            d128 = ppool.tile([128, N_CORES, NW // 2], f32, tag="d128")

            # ---- degree pass -> local dinv (window-slot layout)
            with tc.tile_pool(name="deg", bufs=1) as dpool:
                degt = dpool.tile([SLOTS, NW, DEG_K], f32, tag="degt")
                nc.sync.dma_start(
                    degt[:], ewdeg[:].rearrange("(w s) k -> s w k", s=SLOTS)
                )
                deg = dpool.tile([SLOTS, NW], f32, tag="deg")
                nc.vector.tensor_reduce(
                    out=deg[:], in_=degt[:], axis=mybir.AxisListType.X,
                    op=Alu.add,
                )
                rec = dpool.tile([SLOTS, NW], f32, tag="rec")
                nc.vector.reciprocal(rec[:], deg[:])
                nc.scalar.activation(dinvw[:], rec[:], Act.Sqrt)
                nc.sync.dma_start(
                    dinv_loc_d[:].rearrange("(w s) -> s w", s=SLOTS), dinvw[:]
                )

            # all-gather dinv; load in [128, r, NW/2] chunk layout for preamble
            nc.gpsimd.collective_compute(
                "AllGather", Alu.bypass, replica_groups=groups,
                ins=[dinv_loc_d[:]],
                outs=[dinv_glob_d[:].rearrange("r n -> (r n)")],
            )
            nc.sync.dma_start(
                d128[:],
                dinv_glob_d[:].rearrange("r (c p) -> p r c", p=128),
            )

            # ---- preamble: h0 = dinv * (x @ W1) in bf16 [NV, 128] rows
            PB = 8  # 128-row chunks per h0 write
            h0_writes = []
            with (
                tc.tile_pool(name="pre", bufs=2) as prepool,
                tc.tile_pool(name="prepsum", bufs=4, space="PSUM") as pspool,
                tc.tile_pool(name="prestage", bufs=2) as stpool,
            ):
                for r in range(N_CORES):
                    xts = prepool.tile([IN_F, NVC], bf16, tag="xts")
                    nc.sync.dma_start(xts[:], xt[:, r * NVC : (r + 1) * NVC])
                    for cb in range(NW // 2 // PB):
                        stage = stpool.tile([128, PB, 128], bf16, tag="h0st")
                        nc.vector.memset(stage[:, :, HID:128], 0.0)
                        for j in range(PB):
                            ch = cb * PB + j
                            ps = pspool.tile([128, HID], f32, tag="prepsum")
                            nc.tensor.matmul(
                                out=ps[:],
                                lhsT=xts[:, ch * 128 : (ch + 1) * 128],
                                rhs=w1s[:],
                                start=True, stop=True,
                            )
                            nc.scalar.activation(
                                stage[:, j, 0:HID], ps[:], Act.Copy,
                                scale=d128[:, r, ch : ch + 1],
                            )
                        row0 = r * NVC + cb * PB * 128
                        h0_writes.append(
                            nc.sync.dma_start(
                                h0[row0 : row0 + PB * 128, :].rearrange(
                                    "(b p) h -> p b h", p=128
                                ),
                                stage[:],
                            )
                        )

            fence0 = nc.gpsimd.engine_nop()
            for wi in h0_writes:
                add_dep_helper(fence0.ins, wi.ins,
                               reason="h0 table ready before gathers")

            # ---- message-passing layer (one pass over the edge tiles)
            def layer(l, src_table, fence, pool_ps=None):
                lo_view = src_table[0:NVH, :]
                hi_view = src_table[NVH:NV, :]
                with (
                    tc.tile_pool(name=f"idx{l}", bufs=4) as ipool,
                    tc.tile_pool(name=f"mbuf{l}", bufs=2) as mpool,
                    tc.tile_pool(name=f"sbuf{l}", bufs=2) as spool,
                    tc.tile_pool(name=f"wpsum{l}", bufs=4, space="PSUM") as wpool,
                    tc.tile_pool(name=f"epi{l}", bufs=3) as epool,
                    tc.tile_pool(name=f"p2_{l}", bufs=2, space="PSUM") as p2pool,
                ):
                    for g in range(NG):
                        ilo = ipool.tile([128, IDXC], i16, tag="ilo")
                        nc.sync.dma_start(ilo[:], idxlo[g])
                        ihi = ipool.tile([128, IDXC], i16, tag="ihi")
                        nc.sync.dma_start(ihi[:], idxhi[g])
                        mlo = mpool.tile([128, TPG, 128], bf16, tag="mlo")
                        mhi = mpool.tile([128, TPG, 128], bf16, tag="mhi")
                        glo = nc.gpsimd.dma_gather(
                            mlo[:], lo_view, ilo[:], GI, GI, 128,
                            single_packet=False,
                            queue_num=(2 * g) % NQ,
                        )
                        ghi = nc.gpsimd.dma_gather(
                            mhi[:], hi_view, ihi[:], GI, GI, 128,
                            single_packet=False,
                            queue_num=(2 * g + 1) % NQ,
                        )
                        if fence is not None:
                            add_dep_helper(glo.ins, fence.ins,
                                           reason="gather src table ready")
                            add_dep_helper(ghi.ins, fence.ins,
                                           reason="gather src table ready")
                        slo = spool.tile([128, TPG * SLOTS], bf16, tag="slo")
                        nc.sync.dma_start(
                            slo[:], slo_d[:, g * TPG * SLOTS : (g + 1) * TPG * SLOTS]
                        )
                        shi = spool.tile([128, TPG * SLOTS], bf16, tag="shi")
                        nc.sync.dma_start(
                            shi[:], shi_d[:, g * TPG * SLOTS : (g + 1) * TPG * SLOTS]
                        )
                        for wl in range(GROUP_W):
                            w = g * GROUP_W + wl
                            ps = wpool.tile([SLOTS, HID], f32, tag="wps")
                            k = 0
                            for mb, sb in ((mlo, slo), (mhi, shi)):
                                for ti in range(T_SIDE):
                                    blk = wl * T_SIDE + ti
                                    stile = sb[:, blk * SLOTS : (blk + 1) * SLOTS]
                                    first = k == 0
                                    last = k == 2 * T_SIDE - 1
                                    if l == 1:
                                        nc.tensor.matmul(
                                            out=ps[:], lhsT=stile,
                                            rhs=mb[:, blk, 0:HID],
                                            start=first, stop=last,
                                        )
                                    else:
                                        nc.tensor.matmul(
                                            out=ps[:], lhsT=mb[:, blk, 0:HID],
                                            rhs=stile,
                                            start=first, stop=last,
                                        )
                                    k += 1
                            dv = dinvw[:, w : w + 1]
                            if l == 1:
                                # uh = dv*relu(dv*ps + b1) = relu(dv*(dv*ps + b1))
                                t = epool.tile([SLOTS, HID], f32, tag="t1")
                                nc.scalar.activation(
                                    t[:], ps[:], Act.Copy, scale=dv,
                                )
                                u = epool.tile([SLOTS, HID], f32, tag="u1")
                                nc.vector.tensor_tensor(
                                    out=u[:], in0=t[:], in1=b1s[:], op=Alu.add,
                                )
                                uh = epool.tile([SLOTS, 128], bf16, tag="uh")
                                nc.vector.memset(uh[:, HID:128], 0.0)
                                nc.scalar.activation(
                                    uh[:, 0:HID], u[:], Act.Relu, scale=dv,
                                )
                                nc.sync.dma_start(
                                    h1loc[w * SLOTS : (w + 1) * SLOTS, :],
                                    uh[:],
                                )
                            else:
                                aggT = epool.tile([HID, SLOTS], bf16, tag="aggT")
                                nc.vector.tensor_copy(aggT[:], ps[:])
                                ps2 = p2pool.tile([SLOTS, HID], f32, tag="ps2")
                                nc.tensor.matmul(
                                    out=ps2[:], lhsT=aggT[:], rhs=w2s[:],
                                    start=True, stop=True,
                                )
                                u = epool.tile([SLOTS, HID + 1], bf16, tag="u2")
                                nc.vector.memset(u[:, HID : HID + 1], 1.0)
                                t2 = epool.tile([SLOTS, HID], f32, tag="t2")
                                nc.scalar.activation(
                                    t2[:], ps2[:], Act.Copy, scale=dv,
                                )
                                uf = epool.tile([SLOTS, HID], f32, tag="u2f")
                                nc.vector.tensor_tensor(
                                    out=uf[:], in0=t2[:], in1=b2s[:], op=Alu.add,
                                )
                                nc.scalar.activation(
                                    u[:, 0:HID], uf[:], Act.Relu,
                                )
                                nc.tensor.matmul(
                                    out=pool_ps[:], lhsT=u[:],
                                    rhs=sgs[:, w * SG : (w + 1) * SG],
                                    start=(w == 0),
                                    stop=(w == NW - 1),
                                )
                    if l == 2:
                        pst = epool.tile([HID + 1, SG], f32, tag="pst")
                        nc.vector.tensor_copy(pst[:], pool_ps[:])
                        nc.sync.dma_start(pool_in_d[:], pst[:])

            layer(1, h0, fence0)

            cc_h1 = nc.gpsimd.collective_compute(
                "AllGather", Alu.bypass, replica_groups=groups,
                ins=[h1loc[:].rearrange("a b -> (a b)")],
                outs=[h1glob[:].rearrange("a b -> (a b)")],
            )

            with tc.tile_pool(name="gps", bufs=1, space="PSUM") as gpool:
                pool_ps = gpool.tile([HID + 1, SG], f32, tag="poolps")
                layer(2, h1glob, cc_h1, pool_ps=pool_ps)

            # ---- pooled partial sums -> all-reduce -> final linear
            nc.gpsimd.collective_compute(
                "AllReduce", Alu.add, replica_groups=groups,
                ins=[pool_in_d[:]], outs=[pool_out_d[:]],
            )
            with (
                tc.tile_pool(name="fin", bufs=1) as fpool,
                tc.tile_pool(name="finps", bufs=1, space="PSUM") as fpsum,
            ):
                pr = fpool.tile([HID + 1, SG], f32, tag="pr")
                nc.sync.dma_start(pr[:], pool_out_d[:])
                cm = fpool.tile([1, SG], f32, tag="cm")
                nc.vector.tensor_scalar(
                    out=cm[:], in0=pr[HID : HID + 1, :], scalar1=1.0,
                    scalar2=None, op0=Alu.max,
                )
                rcp = fpool.tile([1, SG], f32, tag="rcp")
                nc.vector.reciprocal(rcp[:], cm[:])
                rcpb = fpool.tile([HID, SG], f32, tag="rcpb")
                nc.gpsimd.partition_broadcast(rcpb[:], rcp[:])
                pooledT = fpool.tile([HID, N_GRAPHS], f32, tag="pooledT")
                nc.vector.tensor_tensor(
                    out=pooledT[:], in0=pr[0:HID, 0:N_GRAPHS],
                    in1=rcpb[0:HID, 0:N_GRAPHS],
                    op=Alu.mult,
                )
                pso = fpsum.tile([N_GRAPHS, OUT_F], f32, tag="pso")
                nc.tensor.matmul(
                    out=pso[:], lhsT=pooledT[:], rhs=wos[:],
                    start=True, stop=True,
                )
                osb = fpool.tile([N_GRAPHS, OUT_F], f32, tag="osb")
                nc.vector.tensor_tensor(
                    out=osb[:], in0=pso[:],
                    in1=bos[:],
                    op=Alu.add,
                )
                nc.sync.dma_start(out[:], osb[:])
                chs = fpool.tile([1, 4], f32, tag="chs")
                nc.sync.dma_start(chs[:], chain_in[:])
                nc.vector.tensor_scalar_add(chs[:], chs[:], 1.0)
                nc.sync.dma_start(chain_out[:], chs[:])

    nc.compile()
    return nc


def kernel(x, edge_index, edge_attr, batch, W1, b1, W2, b2, Wo, bo, **_):
    import ml_dtypes

    bf16 = ml_dtypes.bfloat16

    per_core, plan, xt_virt = _pack_host(x, edge_index, edge_attr, batch)
    nc = _build_program(plan)

    common = dict(
        chain=np.zeros((1, 4), np.float32),
        xt=xt_virt,
        w1=np.asarray(W1, np.float32).astype(bf16),
        w2=np.asarray(W2, np.float32).astype(bf16),
        wo=np.asarray(Wo, np.float32),
        b1=np.tile(np.asarray(b1, np.float32).reshape(1, -1), (SLOTS, 1)),
        b2=np.tile(np.asarray(b2, np.float32).reshape(1, -1), (SLOTS, 1)),
        bo=np.tile(np.asarray(bo, np.float32).reshape(1, -1), (N_GRAPHS, 1)),
    )
    in_maps = []
    for c in range(N_CORES):
        m = dict(common)
        m.update(per_core[c])
        in_maps.append(m)

    from concourse.bass_utils import run_bass_kernel_spmd

    res = run_bass_kernel_spmd(nc, in_maps, list(range(N_CORES)))
    out = res.results[0]["out"]
    kernel.last_exec_time_ns = res.exec_time_ns
    kernel.last_results = res.results
    kernel.last_res = res
    return np.asarray(out, np.float32)


kernel.last_exec_time_ns = None


# revision 25
# speedup vs baseline: 2.7730x; 1.0307x over previous
"""Trainium2 Bass kernel for a 2-layer GCN (FCGraphGNN) over 8 NeuronCores.

Math (matches reference):
  ew' = [edge_attr; ones(N)]  (self loops), deg = segsum(ew', dst), dinv = deg^-1/2
  h1 = relu(segsum(dinv[src]*ew*dinv[dst] * (x@W1)[src]) + b1)
  h2 = relu(segsum(norm * (h1@W2)[src]) + b2)
  out = mean-pool-by-graph(h2) @ Wo + bo

Strategy (v2):
  - Shard edges by dst across 8 cores (sorted by dst host-side).
  - Virtual node space: 64-node windows, NW=128 windows/core, NV=65536.
  - Feature tables (h0 = dinv*(x@W1), h1glob = dinv*h1) stored bf16 as
    [NV, 128] rows (64 real features + 64 zero pad) so each dma_gather
    descriptor moves the minimum 256B.
  - Messages gathered per edge with dma_gather (int16 idx, lo/hi halves).
  - Segment-sum via PE: per 128-edge tile a host-PRECOMPUTED one-hot
    S[128,64] (bf16, entries=edge weight) is loaded from DRAM and
    matmul'd against the gathered messages into PSUM per window.
    (v1 built S on DVE per tile -> DVE was the bottleneck.)
  - dinv computed on device from a padded ewdeg table; dinv + h1
    all-gathered across cores; pooled partial sums all-reduced.
  - Pool one-hot Sg (graph ids) precomputed on host (junk bin absorbs
    unused slots, so no per-core count bookkeeping -> SPMD uniform).
"""

import os
import sys

import numpy as np

sys.path.insert(0, "/opt/trn_rl_repo")

# ---------------------------------------------------------------- constants
N_NODES = 50000
N_EDGES = 3200000
N_GRAPHS = 50
IN_F = 5
HID = 64
OUT_F = 2
N_CORES = 8

SLOTS = 64          # dst nodes per window
T_SIDE = 13         # 128-edge tiles per (window, src-half)
CAP = T_SIDE * 128  # edge slots per (window, side)
NW = 128            # windows per core (uniform across cores)
NVC = NW * SLOTS    # 8192 virtual rows per core
NV = N_CORES * NVC  # 65536
NVH = NV // 2       # 32768 (int16 idx budget per half)
GROUP_W = 4         # windows fetched per dma_gather pair
GI = GROUP_W * CAP  # idxs per gather call (6656)
IDXC = GI // 16
NG = NW // GROUP_W  # 32 groups
TPG = GROUP_W * T_SIDE  # 52 tiles per (group, side)
NT_SIDE = NW * T_SIDE   # 1664 tiles per side
SG = 52             # graph one-hot width (50 graphs + 2 junk bins)


def _pack_host(x, edge_index, edge_attr, batch):
    """Pure index/layout preprocessing (numpy). Returns per-core input dicts
    plus the static plan (DEG_K)."""
    import ml_dtypes

    bf16 = ml_dtypes.bfloat16

    src = np.asarray(edge_index[0], dtype=np.int64)
    dst = np.asarray(edge_index[1], dtype=np.int64)
    ew = np.asarray(edge_attr, dtype=np.float32).reshape(-1)
    loop = np.arange(N_NODES, dtype=np.int64)
    src = np.concatenate([src, loop]).astype(np.int32)
    dst = np.concatenate([dst, loop]).astype(np.int32)
    ew = np.concatenate([ew, np.ones(N_NODES, np.float32)])
    E = src.shape[0]

    deg_cnt = np.bincount(dst, minlength=N_NODES).astype(np.int64)
    node_ptr = np.zeros(N_NODES + 1, np.int64)
    np.cumsum(deg_cnt, out=node_ptr[1:])
    order = np.argsort(dst, kind="stable")

    # core node boundaries balancing edge counts
    cum = node_ptr[1:]
    nb = [0]
    for c in range(1, N_CORES):
        nb.append(int(np.searchsorted(cum, c * E / N_CORES)))
    nb.append(N_NODES)
    nb = np.array(nb, np.int64)
    split_node = int(nb[4])  # src < split_node -> "lo" half of virtual space

    side_lo = src < split_node
    deg_lo = np.bincount(dst[side_lo], minlength=N_NODES).astype(np.int64)
    deg_hi = deg_cnt - deg_lo

    # window packing per core: <=64 nodes AND <=CAP edges per side
    core_windows = []
    for c in range(N_CORES):
        wlist = []
        v = int(nb[c])
        end = int(nb[c + 1])
        while v < end:
            ws = v
            lo = hi = cnt = 0
            while (
                v < end
                and cnt < SLOTS
                and lo + deg_lo[v] <= CAP
                and hi + deg_hi[v] <= CAP
            ):
                lo += int(deg_lo[v])
                hi += int(deg_hi[v])
                cnt += 1
                v += 1
            wlist.append((ws, v))
        assert len(wlist) <= NW, f"core {c}: {len(wlist)} windows > NW={NW}"
        core_windows.append(wlist)

    # vid map (node -> virtual id)
    node_vid = np.zeros(N_NODES, np.int32)
    for c in range(N_CORES):
        for w, (ws, we) in enumerate(core_windows[c]):
            node_vid[ws:we] = c * NVC + w * SLOTS + np.arange(we - ws, dtype=np.int32)

    DEG_K = int(deg_cnt.max())
    DEG_K = (DEG_K + 3) // 4 * 4

    # per-side dst-sorted edge lists + ptrs
    lo_edges = order[side_lo[order]]
    hi_edges = order[~side_lo[order]]
    lo_ptr = np.zeros(N_NODES + 1, np.int64)
    np.cumsum(deg_lo, out=lo_ptr[1:])
    hi_ptr = np.zeros(N_NODES + 1, np.int64)
    np.cumsum(deg_hi, out=hi_ptr[1:])

    vid_src = node_vid[src]

    # global position of each edge within its dst node's sorted run
    col_within = np.empty(E, np.int64)
    ar = np.arange(E, dtype=np.int64)
    col_within[order] = ar - node_ptr[dst[order]]

    # xt in virtual layout (shared by all cores), bf16
    xt_virt = np.zeros((IN_F, NV), np.float32)
    xt_virt[:, node_vid] = np.asarray(x, np.float32).T
    xt_virt = xt_virt.astype(bf16)

    batch_i = np.asarray(batch, np.int64)

    per_core = []
    for c in range(N_CORES):
        wlist = core_windows[c]
        idx_lo = np.zeros((NW, CAP), np.int16)
        idx_hi = np.zeros((NW, CAP), np.int16)
        # S tiles: [side][tile, row, slot] with tile = w*T_SIDE + ti
        s_lo = np.zeros((NT_SIDE, 128, SLOTS), np.float32)
        s_hi = np.zeros((NT_SIDE, 128, SLOTS), np.float32)
        gid = np.full((SLOTS, NW), 50.0, np.float32)
        ewdeg = np.zeros((NVC, DEG_K), np.float32)

        for w, (ws, we) in enumerate(wlist):
            for edges, ptr, idxbuf, sbuf, voff in (
                (lo_edges, lo_ptr, idx_lo, s_lo, 0),
                (hi_edges, hi_ptr, idx_hi, s_hi, NVH),
            ):
                ids = edges[ptr[ws] : ptr[we]]
                n = ids.shape[0]
                idxbuf[w, :n] = (vid_src[ids] - voff).astype(np.int16)
                slot_in = np.arange(n, dtype=np.int64)
                t = w * T_SIDE + slot_in // 128
                r = slot_in % 128
                sbuf[t, r, (dst[ids] - ws)] = ew[ids]
            gid[: we - ws, w] = batch_i[ws:we].astype(np.float32)

        # padded per-node edge weights for the degree pass
        e_lo = int(node_ptr[nb[c]])
        e_hi = int(node_ptr[nb[c + 1]])
        es = order[e_lo:e_hi]
        rows = node_vid[dst[es]] - c * NVC
        ewdeg[rows, col_within[es]] = ew[es]
        rowdeg = np.zeros(NVC, np.int64)
        nr = node_vid[nb[c] : nb[c + 1]] - c * NVC
        rowdeg[nr] = deg_cnt[nb[c] : nb[c + 1]]
        ewdeg[rowdeg == 0, 0] = 1.0

        # pool one-hot: [SLOTS, NW, SG] -> flat [SLOTS, NW*SG] bf16
        sgp = np.zeros((SLOTS, NW, SG), np.float32)
        ss, ww = np.meshgrid(np.arange(SLOTS), np.arange(NW), indexing="ij")
        sgp[ss, ww, gid.astype(np.int64)] = 1.0
        sgp = sgp.reshape(SLOTS, NW * SG).astype(bf16)

        # wrap gather indices: [NG, 128, IDXC] (16-partition wrap, replicated)
        def wrap(a):
            g = a.reshape(NG, GI // 16, 16).transpose(0, 2, 1)  # [NG,16,IDXC]
            return np.ascontiguousarray(np.tile(g, (1, 8, 1)))

        # S tiles in DMA layout: [128, NT_SIDE*SLOTS] bf16
        def slayout(s):
            return np.ascontiguousarray(
                s.transpose(1, 0, 2).reshape(128, NT_SIDE * SLOTS).astype(bf16)
            )

        per_core.append(
            dict(
                slo=slayout(s_lo),
                shi=slayout(s_hi),
                idxlo=wrap(idx_lo.reshape(-1)),
                idxhi=wrap(idx_hi.reshape(-1)),
                sgp=sgp,
                ewdeg=ewdeg,
            )
        )

    plan = dict(DEG_K=DEG_K)
    return per_core, plan, xt_virt


def _build_program(plan):
    import concourse.bacc as bacc
    import concourse.bass as bass
    import concourse.tile as tile
    from concourse import mybir
    from concourse.tile_rust import add_dep_helper

    f32 = mybir.dt.float32
    bf16 = mybir.dt.bfloat16
    i16 = mybir.dt.int16
    Alu = mybir.AluOpType
    Act = mybir.ActivationFunctionType

    DEG_K = plan["DEG_K"]

    NQ = int(os.environ.get("KNQ", "4"))
    nc = bacc.Bacc("TRN2", target_bir_lowering=False, debug=False,
                   num_devices=N_CORES, num_swdge_queues=NQ)

    xt = nc.declare_dram_parameter("xt", [IN_F, NV], bf16, isOutput=False)
    w1 = nc.declare_dram_parameter("w1", [IN_F, HID], bf16, isOutput=False)
    w2 = nc.declare_dram_parameter("w2", [HID, HID], bf16, isOutput=False)
    wo = nc.declare_dram_parameter("wo", [HID, OUT_F], f32, isOutput=False)
    b1 = nc.declare_dram_parameter("b1", [SLOTS, HID], f32, isOutput=False)
    b2 = nc.declare_dram_parameter("b2", [SLOTS, HID], f32, isOutput=False)
    bo = nc.declare_dram_parameter("bo", [N_GRAPHS, OUT_F], f32, isOutput=False)
    ewdeg = nc.declare_dram_parameter("ewdeg", [NVC, DEG_K], f32, isOutput=False)
    slo_d = nc.declare_dram_parameter("slo", [128, NT_SIDE * SLOTS], bf16,
                                      isOutput=False)
    shi_d = nc.declare_dram_parameter("shi", [128, NT_SIDE * SLOTS], bf16,
                                      isOutput=False)
    idxlo = nc.declare_dram_parameter("idxlo", [NG, 128, IDXC], i16, isOutput=False)
    idxhi = nc.declare_dram_parameter("idxhi", [NG, 128, IDXC], i16, isOutput=False)
    sgp_d = nc.declare_dram_parameter("sgp", [SLOTS, NW * SG], bf16, isOutput=False)
    out = nc.declare_dram_parameter("out", [N_GRAPHS, OUT_F], f32, isOutput=True)
    chain_in = nc.declare_dram_parameter("chain", [1, 4], f32, isOutput=False)
    chain_out = nc.declare_dram_parameter("chain_out", [1, 4], f32, isOutput=True)

    groups = [list(range(N_CORES))]

    with tile.TileContext(nc) as tc:
        with (
            tc.tile_pool(name="dram", bufs=1, space="DRAM") as dram,
            tc.tile_pool(name="const", bufs=1) as cpool,
            tc.tile_pool(name="persist", bufs=1) as ppool,
        ):
            h0 = dram.tile([NV, 128], bf16, tag="h0")
            h1loc = dram.tile([NVC, 128], bf16, tag="h1loc")
            h1glob = dram.tile([NV, 128], bf16, tag="h1glob", addr_space="Shared")
            dinv_loc_d = dram.tile([NVC], f32, tag="dinvloc")
            dinv_glob_d = dram.tile([N_CORES, NVC], f32, tag="dinvglob")
            pool_in_d = dram.tile([HID + 1, SG], f32, tag="poolin")
            pool_out_d = dram.tile([HID + 1, SG], f32, tag="poolout")

            # ---- constants
            w1s = cpool.tile([IN_F, HID], bf16, tag="w1s")
            nc.sync.dma_start(w1s[:], w1[:])
            w2s = cpool.tile([HID, HID], bf16, tag="w2s")
            nc.sync.dma_start(w2s[:], w2[:])
            wos = cpool.tile([HID, OUT_F], f32, tag="wos")
            nc.sync.dma_start(wos[:], wo[:])
            b1s = cpool.tile([SLOTS, HID], f32, tag="b1s")
            nc.sync.dma_start(b1s[:], b1[:])
            b2s = cpool.tile([SLOTS, HID], f32, tag="b2s")
            nc.sync.dma_start(b2s[:], b2[:])
            bos = cpool.tile([N_GRAPHS, OUT_F], f32, tag="bos")
            nc.sync.dma_start(bos[:], bo[:])
            sgs = cpool.tile([SLOTS, NW * SG], bf16, tag="sgs")
            nc.sync.dma_start(sgs[:], sgp_d[:])

            dinvw = ppool.tile([SLOTS, NW], f32, tag="dinvw")
            d128 = ppool.tile([128, NV // 128], f32, tag="d128")

            # ---- degree pass -> local dinv (window-slot layout)
            with tc.tile_pool(name="deg", bufs=1) as dpool:
                degt = dpool.tile([SLOTS, NW, DEG_K], f32, tag="degt")
                nc.sync.dma_start(
                    degt[:], ewdeg[:].rearrange("(w s) k -> s w k", s=SLOTS)
                )
                deg = dpool.tile([SLOTS, NW], f32, tag="deg")
                nc.vector.tensor_reduce(
                    out=deg[:], in_=degt[:], axis=mybir.AxisListType.X,
                    op=Alu.add,
                )
                rec = dpool.tile([SLOTS, NW], f32, tag="rec")
                nc.vector.reciprocal(rec[:], deg[:])
                nc.scalar.activation(dinvw[:], rec[:], Act.Sqrt)
                nc.sync.dma_start(
                    dinv_loc_d[:].rearrange("(w s) -> s w", s=SLOTS), dinvw[:]
                )

            # all-gather dinv
            nc.gpsimd.collective_compute(
                "AllGather", Alu.bypass, replica_groups=groups,
                ins=[dinv_loc_d[:]],
                outs=[dinv_glob_d[:].rearrange("r n -> (r n)")],
            )
            nc.sync.dma_start(
                d128[:],
                dinv_glob_d[:].rearrange("r (c p) -> p (r c)", p=128),
            )

            # ---- preamble: h0 = dinv * (x @ W1) in bf16 [NV, 128] rows
            PB = 8  # 128-row chunks per PSUM bank / h0 write
            h0_writes = []
            with (
                tc.tile_pool(name="pre", bufs=1) as prepool,
                tc.tile_pool(name="prepsum", bufs=2, space="PSUM") as pspool,
                tc.tile_pool(name="prestage", bufs=2) as stpool,
            ):
                xts = prepool.tile([IN_F, NV], bf16, tag="xts")
                nc.sync.dma_start(xts[:], xt[:])
                for bank in range(NV // 128 // PB):
                    pp = pspool.tile([128, PB * HID], f32, tag="pp")
                    for j in range(PB):
                        ch = bank * PB + j
                        nc.tensor.matmul(
                            out=pp[:, j * HID : (j + 1) * HID],
                            lhsT=xts[:, ch * 128 : (ch + 1) * 128],
                            rhs=w1s[:],
                            start=True, stop=True,
                        )
                    stage = stpool.tile([128, PB, 128], bf16, tag="h0st")
                    nc.vector.memset(stage[:, :, HID:128], 0.0)
                    for j in range(PB):
                        ch = bank * PB + j
                        nc.scalar.activation(
                            stage[:, j, 0:HID],
                            pp[:, j * HID : (j + 1) * HID],
                            Act.Copy, scale=d128[:, ch : ch + 1],
                        )
                    row0 = bank * PB * 128
                    h0_writes.append(
                        nc.sync.dma_start(
                            h0[row0 : row0 + PB * 128, :].rearrange(
                                "(b p) h -> p b h", p=128
                            ),
                            stage[:],
                        )
                    )

            fence0 = nc.gpsimd.engine_nop()
            for wi in h0_writes:
                add_dep_helper(fence0.ins, wi.ins,
                               reason="h0 table ready before gathers")

            # ---- message-passing layer (one pass over the edge tiles)
            def layer(l, src_table, fence, pool_ps=None):
                lo_view = src_table[0:NVH, :]
                hi_view = src_table[NVH:NV, :]
                with (
                    tc.tile_pool(name=f"idx{l}", bufs=6) as ipool,
                    tc.tile_pool(name=f"mbuf{l}", bufs=3) as mpool,
                    tc.tile_pool(name=f"sbuf{l}", bufs=3) as spool,
                    tc.tile_pool(name=f"wpsum{l}", bufs=4, space="PSUM") as wpool,
                    tc.tile_pool(name=f"epi{l}", bufs=3) as epool,
                    tc.tile_pool(name=f"p2_{l}", bufs=2, space="PSUM") as p2pool,
                ):
                    for g in range(NG):
                        ilo = ipool.tile([128, IDXC], i16, tag="ilo")
                        nc.sync.dma_start(ilo[:], idxlo[g])
                        ihi = ipool.tile([128, IDXC], i16, tag="ihi")
                        nc.sync.dma_start(ihi[:], idxhi[g])
                        mlo = mpool.tile([128, TPG, 128], bf16, tag="mlo")
                        mhi = mpool.tile([128, TPG, 128], bf16, tag="mhi")
                        glo = nc.gpsimd.dma_gather(
                            mlo[:], lo_view, ilo[:], GI, GI, 128,
                            single_packet=False,
                            queue_num=(2 * g) % NQ,
                        )
                        ghi = nc.gpsimd.dma_gather(
                            mhi[:], hi_view, ihi[:], GI, GI, 128,
                            single_packet=False,
                            queue_num=(2 * g + 1) % NQ,
                        )
                        if fence is not None:
                            add_dep_helper(glo.ins, fence.ins,
                                           reason="gather src table ready")
                            add_dep_helper(ghi.ins, fence.ins,
                                           reason="gather src table ready")
                        slo = spool.tile([128, TPG * SLOTS], bf16, tag="slo")
                        nc.sync.dma_start(
                            slo[:], slo_d[:, g * TPG * SLOTS : (g + 1) * TPG * SLOTS]
                        )
                        shi = spool.tile([128, TPG * SLOTS], bf16, tag="shi")
                        nc.sync.dma_start(
                            shi[:], shi_d[:, g * TPG * SLOTS : (g + 1) * TPG * SLOTS]
                        )
                        for wl in range(GROUP_W):
                            w = g * GROUP_W + wl
                            ps = wpool.tile([SLOTS, HID], f32, tag="wps")
                            k = 0
                            for mb, sb in ((mlo, slo), (mhi, shi)):
                                for ti in range(T_SIDE):
                                    blk = wl * T_SIDE + ti
                                    stile = sb[:, blk * SLOTS : (blk + 1) * SLOTS]
                                    first = k == 0
                                    last = k == 2 * T_SIDE - 1
                                    if l == 1:
                                        nc.tensor.matmul(
                                            out=ps[:], lhsT=stile,
                                            rhs=mb[:, blk, 0:HID],
                                            start=first, stop=last,
                                        )
                                    else:
                                        nc.tensor.matmul(
                                            out=ps[:], lhsT=mb[:, blk, 0:HID],
                                            rhs=stile,
                                            start=first, stop=last,
                                        )
                                    k += 1
                            dv = dinvw[:, w : w + 1]
                            if l == 1:
                                # uh = dv*relu(dv*ps + b1) = relu(dv*(dv*ps + b1))
                                t = epool.tile([SLOTS, HID], f32, tag="t1")
                                nc.scalar.activation(
                                    t[:], ps[:], Act.Copy, scale=dv,
                                )
                                u = epool.tile([SLOTS, HID], f32, tag="u1")
                                nc.vector.tensor_tensor(
                                    out=u[:], in0=t[:], in1=b1s[:], op=Alu.add,
                                )
                                uh = epool.tile([SLOTS, 128], bf16, tag="uh")
                                nc.vector.memset(uh[:, HID:128], 0.0)
                                nc.scalar.activation(
                                    uh[:, 0:HID], u[:], Act.Relu, scale=dv,
                                )
                                nc.sync.dma_start(
                                    h1loc[w * SLOTS : (w + 1) * SLOTS, :],
                                    uh[:],
                                )
                            else:
                                aggT = epool.tile([HID, SLOTS], bf16, tag="aggT")
                                nc.vector.tensor_copy(aggT[:], ps[:])
                                ps2 = p2pool.tile([SLOTS, HID], f32, tag="ps2")
                                nc.tensor.matmul(
                                    out=ps2[:], lhsT=aggT[:], rhs=w2s[:],
                                    start=True, stop=True,
                                )
                                u = epool.tile([SLOTS, HID + 1], bf16, tag="u2")
                                nc.vector.memset(u[:, HID : HID + 1], 1.0)
                                t2 = epool.tile([SLOTS, HID], f32, tag="t2")
                                nc.scalar.activation(
                                    t2[:], ps2[:], Act.Copy, scale=dv,
                                )
                                uf = epool.tile([SLOTS, HID], f32, tag="u2f")
                                nc.vector.tensor_tensor(
                                    out=uf[:], in0=t2[:], in1=b2s[:], op=Alu.add,
                                )
                                nc.scalar.activation(
                                    u[:, 0:HID], uf[:], Act.Relu,
                                )
                                nc.tensor.matmul(
                                    out=pool_ps[:], lhsT=u[:],
                                    rhs=sgs[:, w * SG : (w + 1) * SG],
                                    start=(w == 0),
                                    stop=(w == NW - 1),
                                )
                    if l == 2:
                        pst = epool.tile([HID + 1, SG], f32, tag="pst")
                        nc.vector.tensor_copy(pst[:], pool_ps[:])
                        nc.sync.dma_start(pool_in_d[:], pst[:])

            layer(1, h0, fence0)

            cc_h1 = nc.gpsimd.collective_compute(
                "AllGather", Alu.bypass, replica_groups=groups,
                ins=[h1loc[:].rearrange("a b -> (a b)")],
                outs=[h1glob[:].rearrange("a b -> (a b)")],
            )

            with tc.tile_pool(name="gps", bufs=1, space="PSUM") as gpool:
                pool_ps = gpool.tile([HID + 1, SG], f32, tag="poolps")
                layer(2, h1glob, cc_h1, pool_ps=pool_ps)

            # ---- pooled partial sums -> all-reduce -> final linear
            nc.gpsimd.collective_compute(
                "AllReduce", Alu.add, replica_groups=groups,
                ins=[pool_in_d[:]], outs=[pool_out_d[:]],
            )
            with (
                tc.tile_pool(name="fin", bufs=1) as fpool,
                tc.tile_pool(name="finps", bufs=1, space="PSUM") as fpsum,
            ):
                pr = fpool.tile([HID + 1, SG], f32, tag="pr")
                nc.sync.dma_start(pr[:], pool_out_d[:])
                cm = fpool.tile([1, SG], f32, tag="cm")
                nc.vector.tensor_scalar(
                    out=cm[:], in0=pr[HID : HID + 1, :], scalar1=1.0,
                    scalar2=None, op0=Alu.max,
                )
                rcp = fpool.tile([1, SG], f32, tag="rcp")
                nc.vector.reciprocal(rcp[:], cm[:])
                rcpb = fpool.tile([HID, SG], f32, tag="rcpb")
                nc.gpsimd.partition_broadcast(rcpb[:], rcp[:])
                pooledT = fpool.tile([HID, N_GRAPHS], f32, tag="pooledT")
                nc.vector.tensor_tensor(
                    out=pooledT[:], in0=pr[0:HID, 0:N_GRAPHS],
                    in1=rcpb[0:HID, 0:N_GRAPHS],
                    op=Alu.mult,
                )
                pso = fpsum.tile([N_GRAPHS, OUT_F], f32, tag="pso")
                nc.tensor.matmul(
                    out=pso[:], lhsT=pooledT[:], rhs=wos[:],
                    start=True, stop=True,
                )
                osb = fpool.tile([N_GRAPHS, OUT_F], f32, tag="osb")
                nc.vector.tensor_tensor(
                    out=osb[:], in0=pso[:],
                    in1=bos[:],
                    op=Alu.add,
                )
                nc.sync.dma_start(out[:], osb[:])
                chs = fpool.tile([1, 4], f32, tag="chs")
                nc.sync.dma_start(chs[:], chain_in[:])
                nc.vector.tensor_scalar_add(chs[:], chs[:], 1.0)
                nc.sync.dma_start(chain_out[:], chs[:])

    nc.compile()
    return nc


def kernel(x, edge_index, edge_attr, batch, W1, b1, W2, b2, Wo, bo, **_):
    import ml_dtypes

    bf16 = ml_dtypes.bfloat16

    per_core, plan, xt_virt = _pack_host(x, edge_index, edge_attr, batch)
    nc = _build_program(plan)

    common = dict(
        chain=np.zeros((1, 4), np.float32),
        xt=xt_virt,
        w1=np.asarray(W1, np.float32).astype(bf16),
        w2=np.asarray(W2, np.float32).astype(bf16),
        wo=np.asarray(Wo, np.float32),
        b1=np.tile(np.asarray(b1, np.float32).reshape(1, -1), (SLOTS, 1)),
        b2=np.tile(np.asarray(b2, np.float32).reshape(1, -1), (SLOTS, 1)),
        bo=np.tile(np.asarray(bo, np.float32).reshape(1, -1), (N_GRAPHS, 1)),
    )
    in_maps = []
    for c in range(N_CORES):
        m = dict(common)
        m.update(per_core[c])
        in_maps.append(m)

    from concourse.bass_utils import run_bass_kernel_spmd

    res = run_bass_kernel_spmd(nc, in_maps, list(range(N_CORES)))
    out = res.results[0]["out"]
    kernel.last_exec_time_ns = res.exec_time_ns
    kernel.last_results = res.results
    kernel.last_res = res
    return np.asarray(out, np.float32)


kernel.last_exec_time_ns = None


# revision 40
# speedup vs baseline: 2.8107x; 1.0136x over previous
"""Trainium2 Bass kernel for a 2-layer GCN (FCGraphGNN) over 8 NeuronCores.

Math (matches reference):
  ew' = [edge_attr; ones(N)]  (self loops), deg = segsum(ew', dst), dinv = deg^-1/2
  h1 = relu(segsum(dinv[src]*ew*dinv[dst] * (x@W1)[src]) + b1)
  h2 = relu(segsum(norm * (h1@W2)[src]) + b2)
  out = mean-pool-by-graph(h2) @ Wo + bo

Strategy (v2):
  - Shard edges by dst across 8 cores (sorted by dst host-side).
  - Virtual node space: 64-node windows, NW=128 windows/core, NV=65536.
  - Feature tables (h0 = dinv*(x@W1), h1glob = dinv*h1) stored bf16 as
    [NV, 128] rows (64 real features + 64 zero pad) so each dma_gather
    descriptor moves the minimum 256B.
  - Messages gathered per edge with dma_gather (int16 idx, lo/hi halves).
  - Segment-sum via PE: per 128-edge tile a host-PRECOMPUTED one-hot
    S[128,64] (bf16, entries=edge weight) is loaded from DRAM and
    matmul'd against the gathered messages into PSUM per window.
    (v1 built S on DVE per tile -> DVE was the bottleneck.)
  - dinv computed on device from a padded ewdeg table; dinv + h1
    all-gathered across cores; pooled partial sums all-reduced.
  - Pool one-hot Sg (graph ids) precomputed on host (junk bin absorbs
    unused slots, so no per-core count bookkeeping -> SPMD uniform).
"""

import os
import sys

import numpy as np

sys.path.insert(0, "/opt/trn_rl_repo")

# ---------------------------------------------------------------- constants
N_NODES = 50000
N_EDGES = 3200000
N_GRAPHS = 50
IN_F = 5
HID = 64
OUT_F = 2
N_CORES = 8

SLOTS = 64          # dst nodes per window
T_SIDE = 13         # 128-edge tiles per (window, src-half)
CAP = T_SIDE * 128  # edge slots per (window, side)
NW = 128            # windows per core (uniform across cores)
NVC = NW * SLOTS    # 8192 virtual rows per core
NV = N_CORES * NVC  # 65536
NVH = NV // 2       # 32768 (int16 idx budget per half)
GROUP_W = 4         # windows fetched per dma_gather pair
GI = GROUP_W * CAP  # idxs per gather call (6656)
IDXC = GI // 16
NG = NW // GROUP_W  # 32 groups
TPG = GROUP_W * T_SIDE  # 52 tiles per (group, side)
NT_SIDE = NW * T_SIDE   # 1664 tiles per side
SG = 52             # graph one-hot width (50 graphs + 2 junk bins)


def _pack_host(x, edge_index, edge_attr, batch):
    """Pure index/layout preprocessing (numpy). Returns per-core input dicts
    plus the static plan (DEG_K)."""
    import ml_dtypes

    bf16 = ml_dtypes.bfloat16

    src = np.asarray(edge_index[0], dtype=np.int64)
    dst = np.asarray(edge_index[1], dtype=np.int64)
    ew = np.asarray(edge_attr, dtype=np.float32).reshape(-1)
    loop = np.arange(N_NODES, dtype=np.int64)
    src = np.concatenate([src, loop]).astype(np.int32)
    dst = np.concatenate([dst, loop]).astype(np.int32)
    ew = np.concatenate([ew, np.ones(N_NODES, np.float32)])
    E = src.shape[0]

    deg_cnt = np.bincount(dst, minlength=N_NODES).astype(np.int64)
    node_ptr = np.zeros(N_NODES + 1, np.int64)
    np.cumsum(deg_cnt, out=node_ptr[1:])
    order = np.argsort(dst, kind="stable")

    # core node boundaries balancing edge counts
    cum = node_ptr[1:]
    nb = [0]
    for c in range(1, N_CORES):
        nb.append(int(np.searchsorted(cum, c * E / N_CORES)))
    nb.append(N_NODES)
    nb = np.array(nb, np.int64)
    split_node = int(nb[4])  # src < split_node -> "lo" half of virtual space

    side_lo = src < split_node
    deg_lo = np.bincount(dst[side_lo], minlength=N_NODES).astype(np.int64)
    deg_hi = deg_cnt - deg_lo

    # window packing per core: <=64 nodes AND <=CAP edges per side
    core_windows = []
    for c in range(N_CORES):
        wlist = []
        v = int(nb[c])
        end = int(nb[c + 1])
        while v < end:
            ws = v
            lo = hi = cnt = 0
            while (
                v < end
                and cnt < SLOTS
                and lo + deg_lo[v] <= CAP
                and hi + deg_hi[v] <= CAP
            ):
                lo += int(deg_lo[v])
                hi += int(deg_hi[v])
                cnt += 1
                v += 1
            wlist.append((ws, v))
        assert len(wlist) <= NW, f"core {c}: {len(wlist)} windows > NW={NW}"
        core_windows.append(wlist)

    # vid map (node -> virtual id, rank-major: core*NVC + window*64 + slot)
    node_vid = np.zeros(N_NODES, np.int32)
    for c in range(N_CORES):
        for w, (ws, we) in enumerate(core_windows[c]):
            node_vid[ws:we] = c * NVC + w * SLOTS + np.arange(we - ws, dtype=np.int32)

    DEG_K = int(deg_cnt.max())
    DEG_K = (DEG_K + 3) // 4 * 4

    # per-side dst-sorted edge lists + ptrs
    lo_edges = order[side_lo[order]]
    hi_edges = order[~side_lo[order]]
    lo_ptr = np.zeros(N_NODES + 1, np.int64)
    np.cumsum(deg_lo, out=lo_ptr[1:])
    hi_ptr = np.zeros(N_NODES + 1, np.int64)
    np.cumsum(deg_hi, out=hi_ptr[1:])

    vid_src = node_vid[src]

    # global position of each edge within its dst node's sorted run
    col_within = np.empty(E, np.int64)
    ar = np.arange(E, dtype=np.int64)
    col_within[order] = ar - node_ptr[dst[order]]

    # xt in virtual layout (shared by all cores), bf16
    xt_virt = np.zeros((IN_F, NV), np.float32)
    xt_virt[:, node_vid] = np.asarray(x, np.float32).T
    xt_virt = xt_virt.astype(bf16)

    batch_i = np.asarray(batch, np.int64)

    per_core = []
    for c in range(N_CORES):
        wlist = core_windows[c]
        idx_lo = np.zeros((NW, CAP), np.int16)
        idx_hi = np.zeros((NW, CAP), np.int16)
        # S tiles: [side][tile, row, slot] with tile = w*T_SIDE + ti
        s_lo = np.zeros((NT_SIDE, 128, SLOTS), np.float32)
        s_hi = np.zeros((NT_SIDE, 128, SLOTS), np.float32)
        gid = np.full((SLOTS, NW), 50.0, np.float32)
        ewdeg = np.zeros((NVC, DEG_K), np.float32)

        for w, (ws, we) in enumerate(wlist):
            for edges, ptr, idxbuf, sbuf, voff in (
                (lo_edges, lo_ptr, idx_lo, s_lo, 0),
                (hi_edges, hi_ptr, idx_hi, s_hi, NVH),
            ):
                ids = edges[ptr[ws] : ptr[we]]
                n = ids.shape[0]
                idxbuf[w, :n] = (vid_src[ids] - voff).astype(np.int16)
                slot_in = np.arange(n, dtype=np.int64)
                t = w * T_SIDE + slot_in // 128
                r = slot_in % 128
                sbuf[t, r, (dst[ids] - ws)] = ew[ids]
            gid[: we - ws, w] = batch_i[ws:we].astype(np.float32)

        # padded per-node edge weights for the degree pass
        e_lo = int(node_ptr[nb[c]])
        e_hi = int(node_ptr[nb[c + 1]])
        es = order[e_lo:e_hi]
        rows = node_vid[dst[es]] - c * NVC
        ewdeg[rows, col_within[es]] = ew[es]
        rowdeg = np.zeros(NVC, np.int64)
        nr = node_vid[nb[c] : nb[c + 1]] - c * NVC
        rowdeg[nr] = deg_cnt[nb[c] : nb[c + 1]]
        ewdeg[rowdeg == 0, 0] = 1.0

        # pool one-hot: [SLOTS, NW, SG] -> flat [SLOTS, NW*SG] bf16
        sgp = np.zeros((SLOTS, NW, SG), np.float32)
        ss, ww = np.meshgrid(np.arange(SLOTS), np.arange(NW), indexing="ij")
        sgp[ss, ww, gid.astype(np.int64)] = 1.0
        sgp = sgp.reshape(SLOTS, NW * SG).astype(bf16)

        # wrap gather indices: [NG, 128, IDXC] (16-partition wrap, replicated)
        def wrap(a):
            g = a.reshape(NG, GI // 16, 16).transpose(0, 2, 1)  # [NG,16,IDXC]
            return np.ascontiguousarray(np.tile(g, (1, 8, 1)))

        # S tiles in DMA layout: [128, NT_SIDE*SLOTS] bf16
        def slayout(s):
            return np.ascontiguousarray(
                s.transpose(1, 0, 2).reshape(128, NT_SIDE * SLOTS).astype(bf16)
            )

        per_core.append(
            dict(
                slo=slayout(s_lo),
                shi=slayout(s_hi),
                idxlo=wrap(idx_lo.reshape(-1)),
                idxhi=wrap(idx_hi.reshape(-1)),
                sgp=sgp,
                ewdeg=ewdeg,
            )
        )

    plan = dict(DEG_K=DEG_K)
    return per_core, plan, xt_virt


def _build_program(plan):
    import concourse.bacc as bacc
    import concourse.bass as bass
    import concourse.tile as tile
    from concourse import mybir
    from concourse.tile_rust import add_dep_helper

    f32 = mybir.dt.float32
    bf16 = mybir.dt.bfloat16
    i16 = mybir.dt.int16
    Alu = mybir.AluOpType
    Act = mybir.ActivationFunctionType

    DEG_K = plan["DEG_K"]

    NQ = int(os.environ.get("KNQ", "4"))
    nc = bacc.Bacc("TRN2", target_bir_lowering=False, debug=False,
                   num_devices=N_CORES, num_swdge_queues=NQ)

    xt = nc.declare_dram_parameter("xt", [IN_F, NV], bf16, isOutput=False)
    w1 = nc.declare_dram_parameter("w1", [IN_F, HID], bf16, isOutput=False)
    w2 = nc.declare_dram_parameter("w2", [HID, HID], bf16, isOutput=False)
    wo = nc.declare_dram_parameter("wo", [HID, OUT_F], f32, isOutput=False)
    b1 = nc.declare_dram_parameter("b1", [SLOTS, HID], f32, isOutput=False)
    b2 = nc.declare_dram_parameter("b2", [SLOTS, HID], f32, isOutput=False)
    bo = nc.declare_dram_parameter("bo", [N_GRAPHS, OUT_F], f32, isOutput=False)
    ewdeg = nc.declare_dram_parameter("ewdeg", [NVC, DEG_K], f32, isOutput=False)
    slo_d = nc.declare_dram_parameter("slo", [128, NT_SIDE * SLOTS], bf16,
                                      isOutput=False)
    shi_d = nc.declare_dram_parameter("shi", [128, NT_SIDE * SLOTS], bf16,
                                      isOutput=False)
    idxlo = nc.declare_dram_parameter("idxlo", [NG, 128, IDXC], i16, isOutput=False)
    idxhi = nc.declare_dram_parameter("idxhi", [NG, 128, IDXC], i16, isOutput=False)
    sgp_d = nc.declare_dram_parameter("sgp", [SLOTS, NW * SG], bf16, isOutput=False)
    out = nc.declare_dram_parameter("out", [N_GRAPHS, OUT_F], f32, isOutput=True)
    chain_in = nc.declare_dram_parameter("chain", [1, 4], f32, isOutput=False)
    chain_out = nc.declare_dram_parameter("chain_out", [1, 4], f32, isOutput=True)

    groups = [list(range(N_CORES))]

    with tile.TileContext(nc) as tc:
        with (
            tc.tile_pool(name="dram", bufs=1, space="DRAM") as dram,
            tc.tile_pool(name="const", bufs=1) as cpool,
            tc.tile_pool(name="persist", bufs=1) as ppool,
        ):
            h0 = dram.tile([NV, 128], bf16, tag="h0")
            h1loc = dram.tile([NVC, 128], bf16, tag="h1loc")
            h1glob = dram.tile([NV, 128], bf16, tag="h1glob")
            dinv_loc_d = dram.tile([NVC], f32, tag="dinvloc")
            dinv_glob_d = dram.tile([N_CORES, NVC], f32, tag="dinvglob")
            pool_in_d = dram.tile([HID + 1, SG], f32, tag="poolin")
            pool_out_d = dram.tile([HID + 1, SG], f32, tag="poolout")
            pool_inB_d = dram.tile([HID + 1, SG], f32, tag="poolinB")
            pool_outB_d = dram.tile([HID + 1, SG], f32, tag="pooloutB")

            # ---- constants
            w1s = cpool.tile([IN_F, HID], bf16, tag="w1s")
            nc.sync.dma_start(w1s[:], w1[:])
            w2s = cpool.tile([HID, HID], bf16, tag="w2s")
            nc.sync.dma_start(w2s[:], w2[:])
            wos = cpool.tile([HID, OUT_F], f32, tag="wos")
            nc.sync.dma_start(wos[:], wo[:])
            b1s = cpool.tile([SLOTS, HID], f32, tag="b1s")
            nc.sync.dma_start(b1s[:], b1[:])
            b2s = cpool.tile([SLOTS, HID], f32, tag="b2s")
            nc.sync.dma_start(b2s[:], b2[:])
            bos = cpool.tile([N_GRAPHS, OUT_F], f32, tag="bos")
            nc.sync.dma_start(bos[:], bo[:])
            sgs = cpool.tile([SLOTS, NW * SG], bf16, tag="sgs")
            nc.sync.dma_start(sgs[:], sgp_d[:])

            dinvw = ppool.tile([SLOTS, NW], f32, tag="dinvw")
            d128 = ppool.tile([128, NV // 128], f32, tag="d128")

            # ---- degree pass -> local dinv (window-slot layout)
            with tc.tile_pool(name="deg", bufs=1) as dpool:
                degt = dpool.tile([SLOTS, NW, DEG_K], f32, tag="degt")
                nc.sync.dma_start(
                    degt[:], ewdeg[:].rearrange("(w s) k -> s w k", s=SLOTS)
                )
                deg = dpool.tile([SLOTS, NW], f32, tag="deg")
                nc.vector.tensor_reduce(
                    out=deg[:], in_=degt[:], axis=mybir.AxisListType.X,
                    op=Alu.add,
                )
                rec = dpool.tile([SLOTS, NW], f32, tag="rec")
                nc.vector.reciprocal(rec[:], deg[:])
                nc.scalar.activation(dinvw[:], rec[:], Act.Sqrt)
                nc.sync.dma_start(
                    dinv_loc_d[:].rearrange("(w s) -> s w", s=SLOTS), dinvw[:]
                )

            # all-gather dinv
            nc.gpsimd.collective_compute(
                "AllGather", Alu.bypass, replica_groups=groups,
                ins=[dinv_loc_d[:]],
                outs=[dinv_glob_d[:].rearrange("r n -> (r n)")],
            )
            nc.sync.dma_start(
                d128[:],
                dinv_glob_d[:].rearrange("r (c p) -> p (r c)", p=128),
            )

            # ---- preamble: h0 = dinv * (x @ W1) in bf16 [NV, 128] rows
            PB = 8  # 128-row chunks per PSUM bank / h0 write
            h0_writes = []
            with (
                tc.tile_pool(name="pre", bufs=1) as prepool,
                tc.tile_pool(name="prepsum", bufs=2, space="PSUM") as pspool,
                tc.tile_pool(name="prestage", bufs=2) as stpool,
            ):
                xts = prepool.tile([IN_F, NV], bf16, tag="xts")
                nc.sync.dma_start(xts[:], xt[:])
                for bank in range(NV // 128 // PB):
                    pp = pspool.tile([128, PB * HID], f32, tag="pp")
                    for j in range(PB):
                        ch = bank * PB + j
                        nc.tensor.matmul(
                            out=pp[:, j * HID : (j + 1) * HID],
                            lhsT=xts[:, ch * 128 : (ch + 1) * 128],
                            rhs=w1s[:],
                            start=True, stop=True,
                        )
                    stage = stpool.tile([128, PB, 128], bf16, tag="h0st")
                    nc.vector.memset(stage[:, :, HID:128], 0.0)
                    if bank % 2 == 0:
                        for j in range(PB):
                            ch = bank * PB + j
                            nc.scalar.activation(
                                stage[:, j, 0:HID],
                                pp[:, j * HID : (j + 1) * HID],
                                Act.Copy, scale=d128[:, ch : ch + 1],
                            )
                    else:
                        stf = stpool.tile([128, PB * HID], f32, tag="h0stf")
                        for j in range(PB):
                            ch = bank * PB + j
                            nc.vector.tensor_scalar(
                                out=stf[:, j * HID : (j + 1) * HID],
                                in0=pp[:, j * HID : (j + 1) * HID],
                                scalar1=d128[:, ch : ch + 1],
                                scalar2=None, op0=Alu.mult,
                            )
                        nc.vector.tensor_copy(
                            stage[:, :, 0:HID],
                            stf[:].rearrange("p (b h) -> p b h", b=PB),
                        )
                    row0 = bank * PB * 128
                    h0_writes.append(
                        nc.sync.dma_start(
                            h0[row0 : row0 + PB * 128, :].rearrange(
                                "(b p) h -> p b h", p=128
                            ),
                            stage[:],
                        )
                    )

            fence0 = nc.gpsimd.engine_nop()
            for wi in h0_writes:
                add_dep_helper(fence0.ins, wi.ins,
                               reason="h0 table ready before gathers")

            # ---- message-passing layer (one pass over the edge tiles)
            PSPLIT = 96  # pool accumulator A covers windows [0, PSPLIT)

            def layer(l, src_table, fences, pool_ps=None, pool_psB=None):
                lo_view = src_table[0:NVH, :]
                hi_view = src_table[NVH:NV, :]
                with (
                    tc.tile_pool(name=f"idx{l}", bufs=6) as ipool,
                    tc.tile_pool(name=f"mbuf{l}", bufs=4) as mpool,
                    tc.tile_pool(name=f"sbuf{l}", bufs=2) as spool,
                    tc.tile_pool(name=f"wpsum{l}", bufs=4, space="PSUM") as wpool,
                    tc.tile_pool(name=f"epi{l}", bufs=3) as epool,
                    tc.tile_pool(name=f"p2_{l}", bufs=2, space="PSUM") as p2pool,
                ):
                    for g in range(NG):
                        ilo = ipool.tile([128, IDXC], i16, tag="ilo")
                        nc.sync.dma_start(ilo[:], idxlo[g])
                        ihi = ipool.tile([128, IDXC], i16, tag="ihi")
                        nc.sync.dma_start(ihi[:], idxhi[g])
                        mlo = mpool.tile([128, TPG, 128], bf16, tag="mlo")
                        mhi = mpool.tile([128, TPG, 128], bf16, tag="mhi")
                        glo = nc.gpsimd.dma_gather(
                            mlo[:], lo_view, ilo[:], GI, GI, 128,
                            single_packet=False,
                            queue_num=(2 * g) % NQ,
                        )
                        ghi = nc.gpsimd.dma_gather(
                            mhi[:], hi_view, ihi[:], GI, GI, 128,
                            single_packet=False,
                            queue_num=(2 * g + 1) % NQ,
                        )
                        for fence in fences:
                            add_dep_helper(glo.ins, fence.ins,
                                           reason="gather src table ready")
                            add_dep_helper(ghi.ins, fence.ins,
                                           reason="gather src table ready")
                        slo = spool.tile([128, TPG * SLOTS], bf16, tag="slo")
                        nc.sync.dma_start(
                            slo[:], slo_d[:, g * TPG * SLOTS : (g + 1) * TPG * SLOTS]
                        )
                        shi = spool.tile([128, TPG * SLOTS], bf16, tag="shi")
                        nc.sync.dma_start(
                            shi[:], shi_d[:, g * TPG * SLOTS : (g + 1) * TPG * SLOTS]
                        )
                        for wl in range(GROUP_W):
                            w = g * GROUP_W + wl
                            ps = wpool.tile([SLOTS, HID], f32, tag="wps")
                            k = 0
                            for mb, sb in ((mlo, slo), (mhi, shi)):
                                for ti in range(T_SIDE):
                                    blk = wl * T_SIDE + ti
                                    stile = sb[:, blk * SLOTS : (blk + 1) * SLOTS]
                                    first = k == 0
                                    last = k == 2 * T_SIDE - 1
                                    if l == 1:
                                        nc.tensor.matmul(
                                            out=ps[:], lhsT=stile,
                                            rhs=mb[:, blk, 0:HID],
                                            start=first, stop=last,
                                        )
                                    else:
                                        nc.tensor.matmul(
                                            out=ps[:], lhsT=mb[:, blk, 0:HID],
                                            rhs=stile,
                                            start=first, stop=last,
                                        )
                                    k += 1
                            dv = dinvw[:, w : w + 1]
                            if l == 1:
                                # uh = dv*relu(dv*ps + b1) = relu(dv*(dv*ps + b1))
                                t = epool.tile([SLOTS, HID], f32, tag="t1")
                                nc.scalar.activation(
                                    t[:], ps[:], Act.Copy, scale=dv,
                                )
                                u = epool.tile([SLOTS, HID], f32, tag="u1")
                                nc.vector.tensor_tensor(
                                    out=u[:], in0=t[:], in1=b1s[:], op=Alu.add,
                                )
                                uh = epool.tile([SLOTS, 128], bf16, tag="uh")
                                nc.vector.memset(uh[:, HID:128], 0.0)
                                nc.scalar.activation(
                                    uh[:, 0:HID], u[:], Act.Relu, scale=dv,
                                )
                                nc.sync.dma_start(
                                    h1loc[w * SLOTS : (w + 1) * SLOTS, :],
                                    uh[:],
                                )
                            else:
                                aggT = epool.tile([HID, SLOTS], bf16, tag="aggT")
                                nc.vector.tensor_copy(aggT[:], ps[:])
                                ps2 = p2pool.tile([SLOTS, HID], f32, tag="ps2")
                                nc.tensor.matmul(
                                    out=ps2[:], lhsT=aggT[:], rhs=w2s[:],
                                    start=True, stop=True,
                                )
                                u = epool.tile([SLOTS, HID + 1], bf16, tag="u2")
                                nc.vector.memset(u[:, HID : HID + 1], 1.0)
                                t2 = epool.tile([SLOTS, HID], f32, tag="t2")
                                nc.scalar.activation(
                                    t2[:], ps2[:], Act.Copy, scale=dv,
                                )
                                uf = epool.tile([SLOTS, HID], f32, tag="u2f")
                                nc.vector.tensor_tensor(
                                    out=uf[:], in0=t2[:], in1=b2s[:], op=Alu.add,
                                )
                                nc.scalar.activation(
                                    u[:, 0:HID], uf[:], Act.Relu,
                                )
                                tgt = pool_ps if w < PSPLIT else pool_psB
                                nc.tensor.matmul(
                                    out=tgt[:], lhsT=u[:],
                                    rhs=sgs[:, w * SG : (w + 1) * SG],
                                    start=(w in (0, PSPLIT)),
                                    stop=(w in (PSPLIT - 1, NW - 1)),
                                )
                    if l == 2:
                        pst = epool.tile([HID + 1, SG], f32, tag="pst")
                        nc.vector.tensor_copy(pst[:], pool_ps[:])
                        nc.sync.dma_start(pool_in_d[:], pst[:])
                        pstB = epool.tile([HID + 1, SG], f32, tag="pstB")
                        nc.vector.tensor_copy(pstB[:], pool_psB[:])
                        nc.sync.dma_start(pool_inB_d[:], pstB[:])

            layer(1, h0, [fence0])

            cc_h1 = [
                nc.gpsimd.collective_compute(
                    "AllGather", Alu.bypass, replica_groups=groups,
                    ins=[h1loc[:].rearrange("a b -> (a b)")],
                    outs=[h1glob[:].rearrange("a b -> (a b)")],
                )
            ]

            with tc.tile_pool(name="gps", bufs=1, space="PSUM") as gpool:
                pool_ps = gpool.tile([HID + 1, SG], f32, tag="poolpsA")
                pool_psB = gpool.tile([HID + 1, SG], f32, tag="poolpsB")
                layer(2, h1glob, cc_h1, pool_ps=pool_ps, pool_psB=pool_psB)

            # ---- pooled partial sums -> all-reduce -> final linear
            nc.gpsimd.collective_compute(
                "AllReduce", Alu.add, replica_groups=groups,
                ins=[pool_in_d[:]], outs=[pool_out_d[:]],
            )
            nc.gpsimd.collective_compute(
                "AllReduce", Alu.add, replica_groups=groups,
                ins=[pool_inB_d[:]], outs=[pool_outB_d[:]],
            )
            with (
                tc.tile_pool(name="fin", bufs=1) as fpool,
                tc.tile_pool(name="finps", bufs=1, space="PSUM") as fpsum,
            ):
                prA = fpool.tile([HID + 1, SG], f32, tag="prA")
                nc.sync.dma_start(prA[:], pool_out_d[:])
                prB = fpool.tile([HID + 1, SG], f32, tag="prB")
                nc.sync.dma_start(prB[:], pool_outB_d[:])
                pr = fpool.tile([HID + 1, SG], f32, tag="pr")
                nc.vector.tensor_tensor(
                    out=pr[:], in0=prA[:], in1=prB[:], op=Alu.add,
                )
                cm = fpool.tile([1, SG], f32, tag="cm")
                nc.vector.tensor_scalar(
                    out=cm[:], in0=pr[HID : HID + 1, :], scalar1=1.0,
                    scalar2=None, op0=Alu.max,
                )
                rcp = fpool.tile([1, SG], f32, tag="rcp")
                nc.vector.reciprocal(rcp[:], cm[:])
                rcpb = fpool.tile([HID, SG], f32, tag="rcpb")
                nc.gpsimd.partition_broadcast(rcpb[:], rcp[:])
                pooledT = fpool.tile([HID, N_GRAPHS], f32, tag="pooledT")
                nc.vector.tensor_tensor(
                    out=pooledT[:], in0=pr[0:HID, 0:N_GRAPHS],
                    in1=rcpb[0:HID, 0:N_GRAPHS],
                    op=Alu.mult,
                )
                pso = fpsum.tile([N_GRAPHS, OUT_F], f32, tag="pso")
                nc.tensor.matmul(
                    out=pso[:], lhsT=pooledT[:], rhs=wos[:],
                    start=True, stop=True,
                )
                osb = fpool.tile([N_GRAPHS, OUT_F], f32, tag="osb")
                nc.vector.tensor_tensor(
                    out=osb[:], in0=pso[:],
                    in1=bos[:],
                    op=Alu.add,
                )
                nc.sync.dma_start(out[:], osb[:])
                chs = fpool.tile([1, 4], f32, tag="chs")
                nc.sync.dma_start(chs[:], chain_in[:])
                nc.vector.tensor_scalar_add(chs[:], chs[:], 1.0)
                nc.sync.dma_start(chain_out[:], chs[:])

    nc.compile()
    return nc


def kernel(x, edge_index, edge_attr, batch, W1, b1, W2, b2, Wo, bo, **_):
    import ml_dtypes

    bf16 = ml_dtypes.bfloat16

    per_core, plan, xt_virt = _pack_host(x, edge_index, edge_attr, batch)
    nc = _build_program(plan)

    common = dict(
        chain=np.zeros((1, 4), np.float32),
        xt=xt_virt,
        w1=np.asarray(W1, np.float32).astype(bf16),
        w2=np.asarray(W2, np.float32).astype(bf16),
        wo=np.asarray(Wo, np.float32),
        b1=np.tile(np.asarray(b1, np.float32).reshape(1, -1), (SLOTS, 1)),
        b2=np.tile(np.asarray(b2, np.float32).reshape(1, -1), (SLOTS, 1)),
        bo=np.tile(np.asarray(bo, np.float32).reshape(1, -1), (N_GRAPHS, 1)),
    )
    in_maps = []
    for c in range(N_CORES):
        m = dict(common)
        m.update(per_core[c])
        in_maps.append(m)

    from concourse.bass_utils import run_bass_kernel_spmd

    res = run_bass_kernel_spmd(nc, in_maps, list(range(N_CORES)))
    out = res.results[0]["out"]
    kernel.last_exec_time_ns = res.exec_time_ns
    kernel.last_results = res.results
    kernel.last_res = res
    return np.asarray(out, np.float32)


kernel.last_exec_time_ns = None
